# revision 35
# baseline (speedup 1.0000x reference)
"""CARAFE ghost-conv kernel for 8 Trainium2 NeuronCores.

Self-contained: takes FULL inputs (as in setup_inputs()), returns FULL output
(4, 256, 160, 160) float32.

Sharding: 8 cores = 4 batches x 2 H-halves (data-parallel, halo'd on host).
Per core: input rows [40*half-6, 40*half+46) (52 rows, zero-padded outside the
image), W padded 80->84 (cols 2..81 valid). Output rows [80*half, 80*half+80).

Pipeline per core (all resident in SBUF):
  S1 down_cv1 1x1 256->32 (PE) + BN+SiLU (ACT)
  S6 Z = out_cv1-conv(x) at low res (PE; CARAFE fused through the 1x1),
     DMA-transposed to pixel-major ztsall, then 3 dw-shifted copies zts1..3
  S2 down_cv2 dw5x5 (PE diag bf16) + BN+SiLU
  S3 enc_cv1 3x3 64->18 (PE, 9 shifted-tap matmuls) + BN+SiLU
  S4 enc_cv2 dw5x5 (PE diag bf16) + BN+SiLU
  S5 softmax: exp on ACT (channel-major) -> DMA-transpose to pixel-major ktp
     -> k-reduce + recip + normalize (DVE) -> ktn (values duplicated x2)
  S7 CARAFE row-pair rh: ONE DVE tensor_tensor builds 9x4 kt-scaled diagonal
     matrices B (ident x ktn broadcast); 9 PE matmuls (stationary zts row,
     moving B slice) accumulate all 4 phases into PSUM [128,4,80];
     ACT SiLU+BN -> o2c; GPSIMD cast -> o2bf
  S9 out_cv2 dw5x5 on the 160-grid, 3-row chunks interleaved into the S7
     loop, split across PE (diag bf16 matmuls) / DVE / GPSIMD (STT chains)
  S10 channels 0..127 output DMA, chunked + interleaved
"""

import numpy as np
import ml_dtypes

import concourse.bacc as bacc
import concourse.bass as bass
import concourse.tile as tile
from concourse import mybir
from concourse.bass_utils import run_bass_kernel_spmd
from concourse.masks import make_identity

F32 = mybir.dt.float32
F32R = mybir.dt.float32r
F8 = mybir.dt.float8e4
PM = mybir.MatmulPerfMode
BF16 = mybir.dt.bfloat16
AF = mybir.ActivationFunctionType
ALU = mybir.AluOpType
AX = mybir.AxisListType

EPS = 1e-5
WP = 84          # padded low-res width
ROWS = 52        # local input rows (valid image rows at local 6..45)
NKT = 42         # kt / o rows (local rows 5..46)
NZ = 44          # Z rows kept (local rows 4..47)
WO = 164         # padded hi-res width
HO = 84          # hi-res rows (output rows 80*half-2 .. 80*half+82)
WZ = 128         # zc padded width (DMA-transpose needs %128 free dim)

# S9 engine split: chunk index (of 27 3-row chunks) -> engine
S9_DVE = {3, 8, 13, 18}
S9_POOL = set()
S9_TAP_BUDGET = 5       # DVE/Pool taps emitted per rh iteration per engine


def _act(nc, out, in_, func, scale=1.0, bias=0.0):
    nc.scalar.activation(out=out, in_=in_, func=func, scale=scale, bias=bias)


def build_kernel():
    nc = bacc.Bacc("TRN2", target_bir_lowering=False, debug=False, num_devices=8)

    d = {}
    d["x_d"] = nc.declare_dram_parameter("x", [256, ROWS, WP], BF16, isOutput=False)
    d["edge_d"] = nc.declare_dram_parameter("edge", [128, 2], F32, isOutput=False)
    d["wdn1_d"] = nc.declare_dram_parameter("wdn1", [256, 32], BF16, isOutput=False)
    d["bdn1_d"] = nc.declare_dram_parameter("bdn1", [32, 2], F32, isOutput=False)
    d["ddn2_d"] = nc.declare_dram_parameter("ddn2", [25, 128, 128], BF16, isOutput=False)
    d["bdn2_d"] = nc.declare_dram_parameter("bdn2", [128, 2], F32, isOutput=False)
    d["wenc_d"] = nc.declare_dram_parameter("wenc", [9, 64, 18], BF16, isOutput=False)
    d["benc1_d"] = nc.declare_dram_parameter("benc1", [18, 2], F32, isOutput=False)
    d["denc2_d"] = nc.declare_dram_parameter("denc2", [25, 126, 126], BF16, isOutput=False)
    d["benc2_d"] = nc.declare_dram_parameter("benc2", [126, 2], F32, isOutput=False)
    d["wout1_d"] = nc.declare_dram_parameter("wout1", [256, 128], BF16, isOutput=False)
    d["bout1_d"] = nc.declare_dram_parameter("bout1", [128, 2], F32, isOutput=False)
    d["dgf8_d"] = nc.declare_dram_parameter("dgf8", [64, 25, 2, 128], F8, isOutput=False)
    d["bout1h_d"] = nc.declare_dram_parameter("bout1h", [64, 2, 2], F32, isOutput=False)
    d["wto2_d"] = nc.declare_dram_parameter("wto2", [128, 25], F32, isOutput=False)
    d["bout2_d"] = nc.declare_dram_parameter("bout2", [128, 2], F32, isOutput=False)
    d["out1_d"] = nc.declare_dram_parameter("out1", [128, 80, 160], BF16, isOutput=True)
    d["out2_d"] = nc.declare_dram_parameter("out2", [128, 80, 160], F32, isOutput=True)

    with tile.TileContext(nc) as tc:
        _emit(nc, tc, d)
    nc.compile()
    return nc


def _emit(nc, tc, d):
    x_d, out1_d, out2_d = d["x_d"], d["out1_d"], d["out2_d"]

    from contextlib import ExitStack
    ctx = ExitStack()
    with ctx:
        consts = ctx.enter_context(tc.tile_pool(name="consts", bufs=1))
        mid = ctx.enter_context(tc.tile_pool(name="mid", bufs=1))
        stage = ctx.enter_context(tc.tile_pool(name="stage", bufs=3))
        bpool = ctx.enter_context(tc.tile_pool(name="bpool", bufs=3))
        psA = ctx.enter_context(tc.tile_pool(name="psA", bufs=4, space="PSUM"))
        psT = ctx.enter_context(tc.tile_pool(name="psT", bufs=4, space="PSUM"))

        # ---- const tiles --------------------------------------------------
        ident = consts.tile([128, 128], F32)
        identb = consts.tile([128, 128], BF16)
        edge = consts.tile([128, 2], F32)
        wdn1 = consts.tile([128, 2, 32], BF16)
        bdn1 = consts.tile([32, 2], F32)
        ddn2p = consts.tile([128, 25, 128], BF16)
        bdn2 = consts.tile([128, 2], F32)
        wenc = consts.tile([64, 9, 18], BF16)
        benc1 = consts.tile([18, 2], F32)
        denc2p = consts.tile([126, 25, 126], BF16)
        benc2 = consts.tile([126, 2], F32)
        wout1 = consts.tile([128, 2, 128], BF16)
        bout1 = consts.tile([128, 2], F32)
        dgf8 = consts.tile([64, 25, 2, 128], F8)
        bout1h = consts.tile([64, 2, 2], F32)
        wto2 = consts.tile([128, 25], F32)
        bout2 = consts.tile([128, 2], F32)

        # mid-lived tensors
        ktn = mid.tile([128, NKT, 3, 3, 4, 2], BF16)  # [w,rh,dw,dh,r,dup]
        zt1 = mid.tile([84, NZ, 128], BF16)
        zt2 = mid.tile([84, NZ, 128], BF16)
        zt3 = mid.tile([84, NZ, 128], BF16)
        zts = {0: zt1, 1: zt2, 2: zt3}              # dw -> shifted Z (pixel-major)
        et, eb = edge[:, 0:1], edge[:, 1:2]

        with tc.tile_pool(name="early", bufs=1) as early:
            earlyA_cm = tc.tile_pool(name="earlyA", bufs=1)
            earlyA = earlyA_cm.__enter__()
            x0 = earlyA.tile([128, ROWS, WP], BF16)
            x1 = earlyA.tile([128, ROWS, WP], BF16)
            # input DMAs first so S1 can start ASAP
            nc.sync.dma_start(out=x0[:], in_=x_d[0:128])
            # small consts next, big diag weights in stage-use order
            make_identity(nc, ident[:])
            nc.gpsimd.tensor_copy(identb[:], ident[:])
            nc.sync.dma_start(out=edge[:], in_=d["edge_d"][:])
            nc.sync.dma_start(out=wdn1[:, 0, :], in_=d["wdn1_d"][0:128, :])
            nc.sync.dma_start(out=wdn1[:, 1, :], in_=d["wdn1_d"][128:256, :])
            nc.sync.dma_start(out=bdn1[:], in_=d["bdn1_d"][:])
            nc.sync.dma_start(out=wout1[:, 0, :], in_=d["wout1_d"][0:128, :])
            nc.sync.dma_start(out=wout1[:, 1, :], in_=d["wout1_d"][128:256, :])
            nc.sync.dma_start(out=bout1[:], in_=d["bout1_d"][:])
            nc.sync.dma_start(out=x1[:], in_=x_d[128:256])
            nc.sync.dma_start(out=bdn2[:], in_=d["bdn2_d"][:])
            nc.sync.dma_start(out=benc1[:], in_=d["benc1_d"][:])
            nc.sync.dma_start(out=benc2[:], in_=d["benc2_d"][:])

            down_t = early.tile([64, ROWS * WP + 8], BF16)
            down = down_t[:, 4:4 + ROWS * WP].rearrange("p (r w) -> p r w", w=WP)
            e = early.tile([36, ROWS, WP], BF16)
            zc = early.tile([128, NZ, WZ], BF16)
            ztsall = early.tile([128, NZ, 128], BF16)
            ebf = early.tile([48, NKT, 128], BF16)
            ktp = early.tile([128, NKT, 48], BF16)
            s = early.tile([128, NKT, 4], F32)
            nc.gpsimd.memset(down_t[:, 0:4], 0.0)
            nc.gpsimd.memset(down_t[:, 4 + ROWS * WP:], 0.0)
            nc.gpsimd.memset(zc[:, :, WP:WZ], 0.0)
            nc.gpsimd.memset(ebf[32:48, :, :], 0.0)
            nc.gpsimd.memset(ebf[0:36, :, 82:128], 0.0)

            def win(flat, p0, p1, off, rr, w):
                return flat[p0:p1, off:off + rr * w].rearrange(
                    "p (r w) -> p r w", w=w)

            y1 = down[0:32]
            nc.vector.memset(down[32:64, 0:2, :], 0.0)
            nc.vector.memset(down[32:64, 50:52, :], 0.0)
            nc.vector.memset(down[32:64, 2:50, 0:2], 0.0)
            nc.vector.memset(down[32:64, 2:50, 82:84], 0.0)
            nc.vector.memset(y1[:, :, 0:2], 0.0)
            nc.vector.memset(y1[:, :, 82:84], 0.0)

            # ---- S1: down_cv1 + BN + SiLU -------------------------------
            for c0 in range(0, ROWS, 6):
                rr = min(6, ROWS - c0)
                ps = psA.tile([128, 6, WP], F32, tag="ps")
                nc.tensor.matmul(ps[0:32, 0:rr, :], wdn1[:, 0, :],
                                 x0[:, c0:c0 + rr, :], start=True, stop=False)
                nc.tensor.matmul(ps[0:32, 0:rr, :], wdn1[:, 1, :],
                                 x1[:, c0:c0 + rr, :], start=False, stop=True)
                _act(nc, y1[:, c0:c0 + rr, 2:82], ps[0:32, 0:rr, 2:82], AF.Silu,
                     scale=bdn1[:, 0:1], bias=bdn1[:, 1:2])
            nc.vector.tensor_scalar_mul(y1[:, 0:6, :], y1[:, 0:6, :], et[0:32])
            nc.vector.tensor_scalar_mul(y1[:, 46:52, :], y1[:, 46:52, :], eb[0:32])

            # ---- S6: Z (out_cv1 conv, no BN) + pixel-major transform ----
            for c0 in range(0, NZ, 6):
                rr = min(6, NZ - c0)
                ps = psA.tile([128, 6, WP], F32, tag="ps")
                nc.tensor.matmul(ps[:, 0:rr, :], wout1[:, 0, :],
                                 x0[:, 4 + c0:4 + c0 + rr, :], start=True, stop=False)
                nc.tensor.matmul(ps[:, 0:rr, :], wout1[:, 1, :],
                                 x1[:, 4 + c0:4 + c0 + rr, :], start=False, stop=True)
                _act(nc, zc[:, c0:c0 + rr, 0:WP], ps[:, 0:rr, :], AF.Copy)
            earlyA_cm.__exit__(None, None, None)

            nc.gpsimd.dma_start(out=ddn2p[:], in_=d["ddn2_d"][:].rearrange("t k m -> k t m"))
            nc.gpsimd.dma_start(out=wenc[:], in_=d["wenc_d"][:].rearrange("t k m -> k t m"))
            nc.gpsimd.dma_start(out=denc2p[:], in_=d["denc2_d"][:].rearrange("t k m -> k t m"))
            nc.gpsimd.dma_start(out=dgf8[:], in_=d["dgf8_d"][:])
            nc.gpsimd.dma_start(out=wto2[:], in_=d["wto2_d"][:])

            # ---- S2: down_cv2 (diag bf16, 4 row-groups packed) ----------
            # group g (partitions 32g..32g+32) holds y1 rows [12g, 12g+16);
            # its outputs are rows [12g+2, 12g+14)
            y1s_t = early.tile([128, 16 * WP + 8], BF16)
            nc.gpsimd.memset(y1s_t[:, 0:4], 0.0)
            nc.gpsimd.memset(y1s_t[:, 4 + 16 * WP:], 0.0)
            for g in range(4):
                nc.sync.dma_start(
                    out=y1s_t[32 * g:32 * g + 32, 4:4 + 16 * WP],
                    in_=down_t[0:32, 4 + 12 * g * WP:4 + (12 * g + 16) * WP])
            for j in range(2):
                ps = psA.tile([128, 6, WP], F32, tag="ps")
                for t in range(25):
                    dh, dw = divmod(t, 5)
                    off = 4 + (6 * j + dh) * WP + (dw - 2)
                    nc.tensor.matmul(
                        ps[:, 0:6, :], ddn2p[:, t, :],
                        win(y1s_t, 0, 128, off, 6, WP),
                        start=(t == 0), stop=(t == 24))
                for g in range(4):
                    _act(nc,
                         down[32:64, 12 * g + 2 + 6 * j:12 * g + 8 + 6 * j, 2:82],
                         ps[32 * g:32 * g + 32, 0:6, 2:82], AF.Silu,
                         scale=bdn2[32 * g:32 * g + 32, 0:1],
                         bias=bdn2[32 * g:32 * g + 32, 1:2])
                # edge masks: rows 2..5 (j0 g0 rows 0..3, et),
                # rows 46..49 (j1 g3 rows 2..5, eb)
                if j == 0:
                    nc.vector.tensor_scalar_mul(down[32:64, 2:6, :],
                                                down[32:64, 2:6, :], et[32:64])
                else:
                    nc.vector.tensor_scalar_mul(down[32:64, 46:50, :],
                                                down[32:64, 46:50, :], eb[32:64])


            # ---- S3: enc_cv1 (9 taps) + BN + SiLU -----------------------
            e1 = e[0:18]
            e1s_t = early.tile([128, 10 * WP + 8], BF16)
            nc.gpsimd.memset(e1s_t[:, 0:4], 0.0)
            nc.gpsimd.memset(e1s_t[:, 4 + 10 * WP:], 0.0)
            nc.vector.memset(e1[:, :, 0:2], 0.0)
            nc.vector.memset(e1[:, :, 82:84], 0.0)
            nc.vector.memset(e1[:, 2:3, 2:82], 0.0)
            nc.vector.memset(e1[:, 49:50, 2:82], 0.0)
            packed = set()
            for c0 in range(3, 49, 6):
                rr = min(6, 49 - c0)
                ps = psA.tile([128, 6, WP], F32, tag="ps")
                for t in range(9):
                    dh, dw = divmod(t, 3)
                    off = 4 + (c0 - 1 + dh) * WP + (dw - 1)
                    nc.tensor.matmul(
                        ps[0:18, 0:rr, :], wenc[:, t, :],
                        win(down_t, 0, 64, off, rr, WP),
                        start=(t == 0), stop=(t == 8))
                _act(nc, e[0:18, c0:c0 + rr, 2:82], ps[0:18, 0:rr, 2:82], AF.Silu,
                     scale=benc1[:, 0:1], bias=benc1[:, 1:2])
                if c0 == 3:
                    nc.vector.tensor_scalar_mul(e1[:, 3:6, :], e1[:, 3:6, :],
                                                et[0:18])
                if c0 == 45:
                    nc.vector.tensor_scalar_mul(e1[:, 46:49, :], e1[:, 46:49, :],
                                                eb[0:18])
                row_end = c0 + rr if c0 != 45 else 52
                for g in range(7):
                    if g not in packed and 6 * g + 13 <= row_end:
                        packed.add(g)
                        nc.sync.dma_start(
                            out=e1s_t[18 * g:18 * g + 18, 4:4 + 10 * WP],
                            in_=e1[:, 6 * g + 3:6 * g + 13, :])

            # ---- S4: enc_cv2 (diag bf16, 7 row-groups packed) -----------
            # group g (partitions 18g..18g+18) holds e1 rows [6g+3, 6g+13);
            # outputs rows [6g+5, 6g+11)
            ps = psA.tile([128, 6, WP], F32, tag="ps")
            for t in range(25):
                dh, dw = divmod(t, 5)
                off = 4 + dh * WP + (dw - 2)
                nc.tensor.matmul(
                    ps[0:126, 0:6, :], denc2p[:, t, :],
                    win(e1s_t, 0, 126, off, 6, WP),
                    start=(t == 0), stop=(t == 24))
            st = stage.tile([126, 6, WP], BF16, tag="ste2")
            _act(nc, st[:], ps[0:126, :, :], AF.Silu,
                 scale=benc2[:, 0:1], bias=benc2[:, 1:2])
            for g in range(7):
                nc.sync.dma_start(out=e[18:36, 6 * g + 5:6 * g + 11, :],
                                  in_=st[18 * g:18 * g + 18, :, :])

            # ztsall[w, zr, c] = zc[c, zr, w]  (one xbar DMA, 2 chunks);
            # deferred here so the copies don't contend with S2/S3 staging
            nc.scalar.dma_start_transpose(out=ztsall[:, 0:22, :], in_=zc[:, 0:22, :])
            nc.scalar.dma_start_transpose(out=ztsall[:, 22:NZ, :], in_=zc[:, 22:NZ, :])
            # dw-shifted copies at partition base 0 (zts_dw[i] = image col i+dw-1)
            for dw in range(3):
                nc.scalar.dma_start(out=zts[dw][:, :, :], in_=ztsall[dw + 1:dw + 85, :, :])

            # ---- S5: softmax -> ktn (pixel-major, batched) --------------
            # exp in channel-major (pixel j at col j), xbar-transpose to
            # pixel-major, reduce over k, normalize with values dup'd x2.
            RHC = [(0, 2), (2, 6), (8, 10), (18, 12), (30, 12)]
            for r0, rn in RHC:
                _act(nc, ebf[0:36, r0:r0 + rn, 0:82],
                     e[0:36, 5 + r0:5 + r0 + rn, 2:84], AF.Exp)
                nc.sync.dma_start_transpose(out=ktp[:, r0:r0 + rn, :],
                                            in_=ebf[:, r0:r0 + rn, :])
                nc.vector.tensor_reduce(
                    s[0:80, r0:r0 + rn, :],
                    ktp[0:80, r0:r0 + rn, 0:36].rearrange(
                        "w rh (k r) -> w rh r k", k=9),
                    axis=AX.X, op=ALU.add)
                nc.vector.reciprocal(
                    s[0:80, r0:r0 + rn, :].rearrange("w rh r -> w (rh r)"),
                    s[0:80, r0:r0 + rn, :].rearrange("w rh r -> w (rh r)"))
                kv = ktp[0:80, r0:r0 + rn, 0:36].rearrange(
                    "w rh (dh dw r) -> w rh dw dh r", dh=3, dw=3)
                neng = nc.vector if r0 < 12 else nc.gpsimd
                for dw in range(3):
                    for r in range(4):
                        neng.tensor_tensor(
                            ktn[0:80, r0:r0 + rn, dw, :, r, :],
                            kv[:, :, dw, :, r].unsqueeze(3)
                                .to_broadcast((80, rn, 3, 2)),
                            s[0:80, r0:r0 + rn, r:r + 1].unsqueeze(3)
                                .to_broadcast((80, rn, 3, 2)),
                            op=ALU.mult)

            # deferred big weights for phase B (issued last on the queue)
            nc.scalar.dma_start(out=bout1h[:], in_=d["bout1h_d"][:])
            nc.sync.dma_start(out=bout2[:], in_=d["bout2_d"][:])

        # early pool freed here
        with tc.tile_pool(name="late", bufs=1) as late:
            o2bf_t = late.tile([128, HO * WO + 8], BF16)
            o2bf = o2bf_t[:, 4:4 + HO * WO].rearrange("p (r w) -> p r w", w=WO)
            o2f8 = late.tile([64, 2, HO * WO + 8], F8)
            o2f8v = o2f8[:, :, 4:4 + HO * WO].rearrange("p g (r w) -> p g r w", w=WO)
            nc.gpsimd.memset(o2bf_t[:, 0:4], 0.0)
            nc.gpsimd.memset(o2bf_t[:, 4 + HO * WO:], 0.0)
            nc.gpsimd.memset(o2bf[:, :, 0:2], 0.0)
            nc.gpsimd.memset(o2bf[:, :, 162:164], 0.0)
            nc.gpsimd.memset(o2f8[:, :, 0:4], 0.0)
            nc.gpsimd.memset(o2f8[:, :, 4 + HO * WO:], 0.0)
            nc.gpsimd.memset(o2f8v[:, :, :, 0:2], 0.0)
            nc.gpsimd.memset(o2f8v[:, :, :, 162:164], 0.0)

            def win(flat, p0, p1, off, rr, w):
                return flat[p0:p1, off:off + rr * w].rearrange(
                    "p (r w) -> p r w", w=w)

            # S9 chunks: (c0, gr); ready when o2bf rows < c0+gr+2 exist.
            # PE chunks emit whole (matmuls are cheap); DVE/Pool chunks
            # emit S9_TAP_BUDGET taps per rh so B-builds aren't starved.
            qs = {"pe": [], "dve": [], "pool": []}
            ci = 0
            for c0 in range(2, 82, 3):
                gr = min(3, 82 - c0)
                eng = "dve" if ci in S9_DVE else ("pool" if ci in S9_POOL else "pe")
                qs[eng].append((c0, gr))
                ci += 1
            state = {"dve": None, "pool": None}   # in-flight: [c0, gr, t, acc]

            def finish_s9(c0, gr, src):
                st = stage.tile([128, 3, WO], F32, tag="st9")
                _act(nc, st[:, 0:gr, :], src, AF.Silu,
                     scale=bout2[:, 0:1], bias=bout2[:, 1:2])
                nc.sync.dma_start(out=out2_d[:, c0 - 2:c0 - 2 + gr, :],
                                  in_=st[:, 0:gr, 2:162])

            pe_state = {}   # in-flight PE chunk: [c0, gr, dh, ps]

            def step_s9_pe(ready_row):
                """Emit all PE dh-tap-groups whose input rows exist."""
                while True:
                    if not pe_state:
                        if not qs["pe"] or qs["pe"][0][0] + qs["pe"][0][1] - 2 \
                                > ready_row:
                            return
                        c0, gr = qs["pe"].pop(0)
                        ps9 = psA.tile([128, 3, WO], F32, tag="ps")
                        pe_state.update(c0=c0, gr=gr, dh=0, ps=ps9)
                    c0, gr = pe_state["c0"], pe_state["gr"]
                    dh = pe_state["dh"]
                    if c0 + gr - 2 + dh > ready_row:
                        return
                    ps = pe_state["ps"]
                    for dw in range(5):
                        t = dh * 5 + dw
                        off = 4 + (c0 - 2 + dh) * WO + (dw - 2)
                        nc.tensor.matmul(
                            ps[:, 0:gr, :], dgf8[:, t, :, :],
                            o2f8[:, :, off:off + gr * WO].rearrange(
                                "p g (r w) -> p g r w", w=WO),
                            start=(t == 0), stop=(t == 24),
                            perf_mode=PM.DoubleRow)
                    pe_state["dh"] = dh + 1
                    if pe_state["dh"] == 5:
                        finish_s9(c0, gr, ps[:, 0:gr, :])
                        pe_state.clear()

            def step_s9(eng, ready_row, budget):
                """Emit up to `budget` taps of eng's in-flight chunk."""
                v = nc.vector if eng == "dve" else nc.gpsimd
                while budget > 0:
                    if state[eng] is None:
                        if not qs[eng] or qs[eng][0][0] + qs[eng][0][1] + 2 > ready_row:
                            return
                        c0, gr = qs[eng].pop(0)
                        acc = stage.tile([128, 3, WO], F32, tag="acc" + eng)
                        state[eng] = [c0, gr, 0, acc]
                    c0, gr, t, acc = state[eng]
                    n = min(budget, 25 - t)
                    for tt in range(t, t + n):
                        dh, dw = divmod(tt, 5)
                        off = 4 + (c0 - 2 + dh) * WO + (dw - 2)
                        src_w = win(o2bf_t, 0, 128, off, gr, WO)
                        if tt == 0:
                            v.tensor_scalar_mul(acc[:, 0:gr, :], src_w,
                                                wto2[:, 0:1])
                        else:
                            v.scalar_tensor_tensor(
                                out=acc[:, 0:gr, :], in0=src_w,
                                scalar=wto2[:, tt:tt + 1], in1=acc[:, 0:gr, :],
                                op0=ALU.mult, op1=ALU.add)
                    budget -= n
                    state[eng][2] = t + n
                    if state[eng][2] == 25:
                        finish_s9(c0, gr, acc[:, 0:gr, :])
                        state[eng] = None

            next_s10 = 0       # S10 row groups of 8: g0 = 8*next_s10

            # ---- S7 + interleaved S9/S10 --------------------------------
            for rh in range(NKT):
                # build all 36 kt-scaled diagonal matrices in one DVE op:
                # B[w, dw, dh, r, wo] = ident[w, wo] * ktn[w, rh, 3*dh+dw, r]
                B = bpool.tile([128, 3, 3, 4, 80], BF16, tag="B")
                for dw in range(3):
                    eng = nc.gpsimd if (dw == 2 and rh >= 8) else nc.vector
                    eng.tensor_tensor(
                        B[0:80, dw].rearrange("w dh r (wp q) -> w (dh r) wp q", q=2),
                        identb[0:80, 0:80].rearrange("w (wp q) -> w wp q", q=2)
                            .unsqueeze(1).to_broadcast((80, 12, 40, 2)),
                        ktn[0:80, rh, dw].rearrange("w dh r q -> w (dh r) q")
                            .unsqueeze(2).to_broadcast((80, 12, 40, 2)),
                        op=ALU.mult)
                pt = psT.tile([128, 4, 80], F32, tag="pt")
                k = 0
                for dw in range(3):
                    for dh in range(3):
                        nc.tensor.matmul(
                            pt[:, :, :], zts[dw][0:80, rh + dh, :],
                            B[0:80, dw, dh, :, :],
                            start=(k == 0), stop=(k == 8))
                        k += 1
                _act(nc,
                     o2bf[:, 2 * rh:2 * rh + 2, 2:162].rearrange(
                         "p a (w b) -> p a b w", b=2),
                     pt[:, :, :].rearrange("p (a b) w -> p a b w", b=2),
                     AF.Silu,
                     scale=bout1[:, 0:1], bias=bout1[:, 1:2])
                for g in range(2):
                    _act(nc,
                         o2f8v[:, g, 2 * rh:2 * rh + 2, 2:162].rearrange(
                             "p a (w b) -> p a b w", b=2),
                         pt[64 * g:64 * g + 64, :, :].rearrange(
                             "p (a b) w -> p a b w", b=2),
                         AF.Silu,
                         scale=bout1h[:, g, 0:1], bias=bout1h[:, g, 1:2])
                if rh == 0:
                    nc.vector.tensor_scalar_mul(o2bf[:, 0:2, :], o2bf[:, 0:2, :], et)
                    for g in range(2):
                        nc.vector.tensor_copy(o2f8v[:, g, 0:2, :],
                                              o2bf[64 * g:64 * g + 64, 0:2, :])
                if rh == NKT - 1:
                    nc.vector.tensor_scalar_mul(o2bf[:, 82:84, :], o2bf[:, 82:84, :], eb)
                    for g in range(2):
                        nc.vector.tensor_copy(o2f8v[:, g, 82:84, :],
                                              o2bf[64 * g:64 * g + 64, 82:84, :])

                # interleave ready S9 work / S10 output DMAs
                ready_row = 2 * rh + 2
                step_s9_pe(ready_row)
                step_s9("dve", ready_row, S9_TAP_BUDGET)
                step_s9("pool", ready_row, S9_TAP_BUDGET)
                while next_s10 < 10 and 8 * next_s10 + 10 <= ready_row:
                    g0 = 8 * next_s10
                    nc.sync.dma_start(out=out1_d[:, g0:g0 + 8, :],
                                      in_=o2bf[:, 2 + g0:10 + g0, 2:162])
                    next_s10 += 1

            # drain remaining S9 work
            while qs["pe"] or pe_state:
                step_s9_pe(HO)
            while qs["dve"] or state["dve"] is not None:
                step_s9("dve", 84, 25)
            while qs["pool"] or state["pool"] is not None:
                step_s9("pool", 84, 25)
            while next_s10 < 10:
                g0 = 8 * next_s10
                nc.sync.dma_start(out=out1_d[:, g0:g0 + 8, :],
                                  in_=o2bf[:, 2 + g0:10 + g0, 2:162])
                next_s10 += 1


# ---------------------------------------------------------------------------
# host side
# ---------------------------------------------------------------------------

_NC_CACHE = {}


def _get_nc():
    if "nc" not in _NC_CACHE:
        _NC_CACHE["nc"] = build_kernel()
    return _NC_CACHE["nc"]


def _bn2(g, b, m, v):
    inv = (g / np.sqrt(v + EPS)).astype(np.float32)
    beta = (b - m * inv).astype(np.float32)
    return np.stack([inv, beta], axis=1).astype(np.float32)


def _diag_taps(w, c, rep=1):
    taps = np.tile(w.reshape(c, 25).T, (1, rep))      # (25, c*rep)
    n = c * rep
    out = np.zeros((25, n, n), np.float32)
    idx = np.arange(n)
    out[:, idx, idx] = taps
    return out.astype(ml_dtypes.bfloat16)


def _tile_bn(bn, rep):
    return np.tile(bn, (rep, 1))


def _wenc_pair(w):
    # taps (dh, dw) of the 3x3 conv; [0:64, dh] = (dh, 0), [64:128, dh] = (dh, 1),
    # [64:128, 3+dh] = (dh, 2) for the shifted-replica pairing in S3
    t = w.reshape(18, 64, 3, 3).transpose(2, 3, 1, 0)   # (dh, dw, c, m)
    out = np.zeros((128, 6, 18), np.float32)
    for dh in range(3):
        out[0:64, dh] = t[dh, 0]
        out[64:128, dh] = t[dh, 1]
        out[64:128, 3 + dh] = t[dh, 2]
    return out.astype(ml_dtypes.bfloat16)


def _dgf8(w):
    taps = w.reshape(128, 25).astype(np.float32)
    out = np.zeros((64, 25, 2, 128), np.float32)
    j = np.arange(64)
    for g in range(2):
        out[j[:, None], np.arange(25)[None, :], g, (64 * g + j)[:, None]] = \
            taps[64 * g:64 * g + 64, :]
    return out.astype(ml_dtypes.float8_e4m3)


def prep_in_maps(inputs):
    inp = {k: np.asarray(v) for k, v in inputs.items()}
    x = inp["x"].astype(np.float32)

    common = dict(
        wdn1=np.ascontiguousarray(inp["down_cv1_w"].reshape(32, 256).T).astype(ml_dtypes.bfloat16),
        bdn1=_bn2(inp["down_cv1_g"], inp["down_cv1_b"], inp["down_cv1_m"], inp["down_cv1_v"]),
        ddn2=_diag_taps(inp["down_cv2_w"], 32, rep=4),
        bdn2=_tile_bn(_bn2(inp["down_cv2_g"], inp["down_cv2_b"], inp["down_cv2_m"], inp["down_cv2_v"]), 4),
        wenc=np.ascontiguousarray(inp["enc_cv1_w"].reshape(18, 64, 9).transpose(2, 1, 0)).astype(ml_dtypes.bfloat16),
        benc1=_bn2(inp["enc_cv1_g"], inp["enc_cv1_b"], inp["enc_cv1_m"], inp["enc_cv1_v"]),
        denc2=_diag_taps(inp["enc_cv2_w"], 18, rep=7),
        benc2=_tile_bn(_bn2(inp["enc_cv2_g"], inp["enc_cv2_b"], inp["enc_cv2_m"], inp["enc_cv2_v"]), 7),
        wout1=np.ascontiguousarray(inp["out_cv1_w"].reshape(128, 256).T).astype(ml_dtypes.bfloat16),
        bout1=_bn2(inp["out_cv1_g"], inp["out_cv1_b"], inp["out_cv1_m"], inp["out_cv1_v"]),
        dgf8=_dgf8(inp["out_cv2_w"]),
        bout1h=_bn2(inp["out_cv1_g"], inp["out_cv1_b"], inp["out_cv1_m"], inp["out_cv1_v"])
            .reshape(2, 64, 2).transpose(1, 0, 2).copy(),
        wto2=np.ascontiguousarray(inp["out_cv2_w"].reshape(128, 25)).astype(np.float32),
        bout2=_bn2(inp["out_cv2_g"], inp["out_cv2_b"], inp["out_cv2_m"], inp["out_cv2_v"]),
    )

    in_maps = []
    for sid in range(8):
        n, half = sid // 2, sid % 2
        h0 = 40 * half
        xs = np.zeros((256, ROWS, WP), ml_dtypes.bfloat16)
        src_lo = max(0, h0 - 6)
        src_hi = min(80, h0 + 46)
        xs[:, src_lo - (h0 - 6):src_hi - (h0 - 6), 2:82] = x[n, :, src_lo:src_hi, :]
        edge = np.zeros((128, 2), np.float32)
        edge[:, 0] = 0.0 if half == 0 else 1.0
        edge[:, 1] = 1.0 if half == 0 else 0.0
        in_maps.append(dict(x=xs, edge=edge, **common))
    return in_maps


def kernel(**inputs):
    in_maps = prep_in_maps(inputs)
    nc = _get_nc()
    res = run_bass_kernel_spmd(nc, in_maps, list(range(8)))
    _NC_CACHE["last_result"] = res

    out = np.empty((4, 256, 160, 160), np.float32)
    for sid in range(8):
        n, half = sid // 2, sid % 2
        out[n, 0:128, 80 * half:80 * half + 80, :] = \
            res.results[sid]["out1"].astype(np.float32)
        out[n, 128:256, 80 * half:80 * half + 80, :] = res.results[sid]["out2"]
    return out


# revision 40
# speedup vs baseline: 1.0109x; 1.0109x over previous
"""CARAFE ghost-conv kernel for 8 Trainium2 NeuronCores.

Self-contained: takes FULL inputs (as in setup_inputs()), returns FULL output
(4, 256, 160, 160) float32.

Sharding: 8 cores = 4 batches x 2 H-halves (data-parallel, halo'd on host).
Per core: input rows [40*half-6, 40*half+46) (52 rows, zero-padded outside the
image), W padded 80->84 (cols 2..81 valid). Output rows [80*half, 80*half+80).

Pipeline per core (all resident in SBUF):
  S1 down_cv1 1x1 256->32 (PE) + BN+SiLU (ACT)
  S6 Z = out_cv1-conv(x) at low res (PE; CARAFE fused through the 1x1),
     DMA-transposed to pixel-major ztsall, then 3 dw-shifted copies zts1..3
  S2 down_cv2 dw5x5 (PE diag bf16) + BN+SiLU
  S3 enc_cv1 3x3 64->18 (PE, 9 shifted-tap matmuls) + BN+SiLU
  S4 enc_cv2 dw5x5 (PE diag bf16) + BN+SiLU
  S5 softmax: exp on ACT (channel-major) -> DMA-transpose to pixel-major ktp
     -> k-reduce + recip + normalize (DVE) -> ktn (values duplicated x2)
  S7 CARAFE row-pair rh: ONE DVE tensor_tensor builds 9x4 kt-scaled diagonal
     matrices B (ident x ktn broadcast); 9 PE matmuls (stationary zts row,
     moving B slice) accumulate all 4 phases into PSUM [128,4,80];
     ACT SiLU+BN -> o2c; GPSIMD cast -> o2bf
  S9 out_cv2 dw5x5 on the 160-grid, 3-row chunks interleaved into the S7
     loop, split across PE (diag bf16 matmuls) / DVE / GPSIMD (STT chains)
  S10 channels 0..127 output DMA, chunked + interleaved
"""

import numpy as np
import ml_dtypes

import concourse.bacc as bacc
import concourse.bass as bass
import concourse.tile as tile
from concourse import mybir
from concourse.bass_utils import run_bass_kernel_spmd
from concourse.masks import make_identity

F32 = mybir.dt.float32
F32R = mybir.dt.float32r
F8 = mybir.dt.float8e4
PM = mybir.MatmulPerfMode
BF16 = mybir.dt.bfloat16
AF = mybir.ActivationFunctionType
ALU = mybir.AluOpType
AX = mybir.AxisListType

EPS = 1e-5
WP = 84          # padded low-res width
ROWS = 52        # local input rows (valid image rows at local 6..45)
NKT = 42         # kt / o rows (local rows 5..46)
NZ = 44          # Z rows kept (local rows 4..47)
WO = 164         # padded hi-res width
HO = 84          # hi-res rows (output rows 80*half-2 .. 80*half+82)
WZ = 128         # zc padded width (DMA-transpose needs %128 free dim)

# S9 engine split: chunk index (of 27 3-row chunks) -> engine
S9_DVE = {3, 8, 13, 18}
S9_POOL = set()
S9_TAP_BUDGET = 3       # DVE/Pool taps emitted per rh iteration per engine


def _act(nc, out, in_, func, scale=1.0, bias=0.0):
    nc.scalar.activation(out=out, in_=in_, func=func, scale=scale, bias=bias)


def build_kernel():
    nc = bacc.Bacc("TRN2", target_bir_lowering=False, debug=False, num_devices=8)

    d = {}
    d["x_d"] = nc.declare_dram_parameter("x", [256, ROWS, WP], BF16, isOutput=False)
    d["edge_d"] = nc.declare_dram_parameter("edge", [128, 2], F32, isOutput=False)
    d["wdn1_d"] = nc.declare_dram_parameter("wdn1", [256, 32], BF16, isOutput=False)
    d["bdn1_d"] = nc.declare_dram_parameter("bdn1", [32, 2], F32, isOutput=False)
    d["ddn2_d"] = nc.declare_dram_parameter("ddn2", [25, 128, 128], BF16, isOutput=False)
    d["bdn2_d"] = nc.declare_dram_parameter("bdn2", [128, 2], F32, isOutput=False)
    d["wenc_d"] = nc.declare_dram_parameter("wenc", [9, 64, 18], BF16, isOutput=False)
    d["benc1_d"] = nc.declare_dram_parameter("benc1", [18, 2], F32, isOutput=False)
    d["denc2_d"] = nc.declare_dram_parameter("denc2", [25, 126, 126], BF16, isOutput=False)
    d["benc2_d"] = nc.declare_dram_parameter("benc2", [126, 2], F32, isOutput=False)
    d["wout1_d"] = nc.declare_dram_parameter("wout1", [256, 128], BF16, isOutput=False)
    d["bout1_d"] = nc.declare_dram_parameter("bout1", [128, 2], F32, isOutput=False)
    d["dgf8_d"] = nc.declare_dram_parameter("dgf8", [64, 25, 2, 128], F8, isOutput=False)
    d["bout1h_d"] = nc.declare_dram_parameter("bout1h", [64, 2, 2], F32, isOutput=False)
    d["wto2_d"] = nc.declare_dram_parameter("wto2", [128, 25], F32, isOutput=False)
    d["bout2_d"] = nc.declare_dram_parameter("bout2", [128, 2], F32, isOutput=False)
    d["out1_d"] = nc.declare_dram_parameter("out1", [128, 80, 160], BF16, isOutput=True)
    d["out2_d"] = nc.declare_dram_parameter("out2", [128, 80, 160], F32, isOutput=True)

    with tile.TileContext(nc) as tc:
        _emit(nc, tc, d)
    nc.compile()
    return nc


def _emit(nc, tc, d):
    x_d, out1_d, out2_d = d["x_d"], d["out1_d"], d["out2_d"]

    from contextlib import ExitStack
    ctx = ExitStack()
    with ctx:
        consts = ctx.enter_context(tc.tile_pool(name="consts", bufs=1))
        mid = ctx.enter_context(tc.tile_pool(name="mid", bufs=1))
        stage = ctx.enter_context(tc.tile_pool(name="stage", bufs=3))
        bpool = ctx.enter_context(tc.tile_pool(name="bpool", bufs=3))
        psA = ctx.enter_context(tc.tile_pool(name="psA", bufs=4, space="PSUM"))
        psT = ctx.enter_context(tc.tile_pool(name="psT", bufs=4, space="PSUM"))

        # ---- const tiles --------------------------------------------------
        ident = consts.tile([128, 128], F32)
        identb = consts.tile([128, 128], BF16)
        edge = consts.tile([128, 2], F32)
        wdn1 = consts.tile([128, 2, 32], BF16)
        bdn1 = consts.tile([32, 2], F32)
        ddn2p = consts.tile([128, 25, 128], BF16)
        bdn2 = consts.tile([128, 2], F32)
        wenc = consts.tile([64, 9, 18], BF16)
        benc1 = consts.tile([18, 2], F32)
        denc2p = consts.tile([126, 25, 126], BF16)
        benc2 = consts.tile([126, 2], F32)
        wout1 = consts.tile([128, 2, 128], BF16)
        bout1 = consts.tile([128, 2], F32)
        dgf8 = consts.tile([64, 25, 2, 128], F8)
        bout1h = consts.tile([64, 2, 2], F32)
        wto2 = consts.tile([128, 25], F32)
        bout2 = consts.tile([128, 2], F32)

        # mid-lived tensors
        ktn = mid.tile([128, NKT, 3, 3, 4, 2], BF16)  # [w,rh,dw,dh,r,dup]
        zt1 = mid.tile([84, NZ, 128], BF16)
        zt2 = mid.tile([84, NZ, 128], BF16)
        zt3 = mid.tile([84, NZ, 128], BF16)
        zts = {0: zt1, 1: zt2, 2: zt3}              # dw -> shifted Z (pixel-major)
        et, eb = edge[:, 0:1], edge[:, 1:2]

        with tc.tile_pool(name="early", bufs=1) as early:
            earlyA_cm = tc.tile_pool(name="earlyA", bufs=1)
            earlyA = earlyA_cm.__enter__()
            x0 = earlyA.tile([128, ROWS, WP], BF16)
            x1 = earlyA.tile([128, ROWS, WP], BF16)
            # input DMAs first so S1 can start ASAP
            nc.sync.dma_start(out=x0[:], in_=x_d[0:128])
            # small consts next, big diag weights in stage-use order
            make_identity(nc, ident[:])
            nc.gpsimd.tensor_copy(identb[:], ident[:])
            nc.sync.dma_start(out=edge[:], in_=d["edge_d"][:])
            nc.sync.dma_start(out=wdn1[:, 0, :], in_=d["wdn1_d"][0:128, :])
            nc.sync.dma_start(out=wdn1[:, 1, :], in_=d["wdn1_d"][128:256, :])
            nc.sync.dma_start(out=bdn1[:], in_=d["bdn1_d"][:])
            nc.sync.dma_start(out=wout1[:, 0, :], in_=d["wout1_d"][0:128, :])
            nc.sync.dma_start(out=wout1[:, 1, :], in_=d["wout1_d"][128:256, :])
            nc.sync.dma_start(out=bout1[:], in_=d["bout1_d"][:])
            nc.sync.dma_start(out=x1[:], in_=x_d[128:256])
            nc.sync.dma_start(out=bdn2[:], in_=d["bdn2_d"][:])
            nc.sync.dma_start(out=benc1[:], in_=d["benc1_d"][:])
            nc.sync.dma_start(out=benc2[:], in_=d["benc2_d"][:])

            down_t = early.tile([64, ROWS * WP + 8], BF16)
            down = down_t[:, 4:4 + ROWS * WP].rearrange("p (r w) -> p r w", w=WP)
            e = early.tile([36, ROWS, WP], BF16)
            zc = early.tile([128, NZ, WZ], BF16)
            ztsall = early.tile([128, NZ, 128], BF16)
            ebf = early.tile([48, NKT, 128], BF16)
            ktp = early.tile([128, NKT, 48], BF16)
            s = early.tile([128, NKT, 4], F32)
            nc.gpsimd.memset(down_t[:, 0:4], 0.0)
            nc.gpsimd.memset(down_t[:, 4 + ROWS * WP:], 0.0)
            nc.gpsimd.memset(zc[:, :, WP:WZ], 0.0)
            nc.gpsimd.memset(ebf[32:48, :, :], 0.0)
            nc.gpsimd.memset(ebf[0:36, :, 82:128], 0.0)

            def win(flat, p0, p1, off, rr, w):
                return flat[p0:p1, off:off + rr * w].rearrange(
                    "p (r w) -> p r w", w=w)

            y1 = down[0:32]
            nc.vector.memset(down[32:64, 0:2, :], 0.0)
            nc.vector.memset(down[32:64, 50:52, :], 0.0)
            nc.vector.memset(down[32:64, 2:50, 0:2], 0.0)
            nc.vector.memset(down[32:64, 2:50, 82:84], 0.0)
            nc.vector.memset(y1[:, :, 0:2], 0.0)
            nc.vector.memset(y1[:, :, 82:84], 0.0)

            # ---- S1: down_cv1 + BN + SiLU -------------------------------
            for c0 in range(0, ROWS, 6):
                rr = min(6, ROWS - c0)
                ps = psA.tile([128, 6, WP], F32, tag="ps")
                nc.tensor.matmul(ps[0:32, 0:rr, :], wdn1[:, 0, :],
                                 x0[:, c0:c0 + rr, :], start=True, stop=False)
                nc.tensor.matmul(ps[0:32, 0:rr, :], wdn1[:, 1, :],
                                 x1[:, c0:c0 + rr, :], start=False, stop=True)
                _act(nc, y1[:, c0:c0 + rr, 2:82], ps[0:32, 0:rr, 2:82], AF.Silu,
                     scale=bdn1[:, 0:1], bias=bdn1[:, 1:2])
            nc.vector.tensor_scalar_mul(y1[:, 0:6, :], y1[:, 0:6, :], et[0:32])
            nc.vector.tensor_scalar_mul(y1[:, 46:52, :], y1[:, 46:52, :], eb[0:32])

            # ---- S6: Z (out_cv1 conv, no BN) + pixel-major transform ----
            for c0 in range(0, NZ, 6):
                rr = min(6, NZ - c0)
                ps = psA.tile([128, 6, WP], F32, tag="ps")
                nc.tensor.matmul(ps[:, 0:rr, :], wout1[:, 0, :],
                                 x0[:, 4 + c0:4 + c0 + rr, :], start=True, stop=False)
                nc.tensor.matmul(ps[:, 0:rr, :], wout1[:, 1, :],
                                 x1[:, 4 + c0:4 + c0 + rr, :], start=False, stop=True)
                _act(nc, zc[:, c0:c0 + rr, 0:WP], ps[:, 0:rr, :], AF.Copy)
            earlyA_cm.__exit__(None, None, None)

            nc.gpsimd.dma_start(out=ddn2p[:], in_=d["ddn2_d"][:].rearrange("t k m -> k t m"))
            nc.gpsimd.dma_start(out=wenc[:], in_=d["wenc_d"][:].rearrange("t k m -> k t m"))
            nc.gpsimd.dma_start(out=denc2p[:], in_=d["denc2_d"][:].rearrange("t k m -> k t m"))
            nc.gpsimd.dma_start(out=dgf8[:], in_=d["dgf8_d"][:])
            nc.gpsimd.dma_start(out=wto2[:], in_=d["wto2_d"][:])

            # ---- S2: down_cv2 (diag bf16, 4 row-groups packed) ----------
            # group g (partitions 32g..32g+32) holds y1 rows [12g, 12g+16);
            # its outputs are rows [12g+2, 12g+14)
            y1s_t = early.tile([128, 16 * WP + 8], BF16)
            nc.gpsimd.memset(y1s_t[:, 0:4], 0.0)
            nc.gpsimd.memset(y1s_t[:, 4 + 16 * WP:], 0.0)
            for g in range(4):
                nc.sync.dma_start(
                    out=y1s_t[32 * g:32 * g + 32, 4:4 + 16 * WP],
                    in_=down_t[0:32, 4 + 12 * g * WP:4 + (12 * g + 16) * WP])
            for j in range(2):
                ps = psA.tile([128, 6, WP], F32, tag="ps")
                for t in range(25):
                    dh, dw = divmod(t, 5)
                    off = 4 + (6 * j + dh) * WP + (dw - 2)
                    nc.tensor.matmul(
                        ps[:, 0:6, :], ddn2p[:, t, :],
                        win(y1s_t, 0, 128, off, 6, WP),
                        start=(t == 0), stop=(t == 24))
                for g in range(4):
                    _act(nc,
                         down[32:64, 12 * g + 2 + 6 * j:12 * g + 8 + 6 * j, 2:82],
                         ps[32 * g:32 * g + 32, 0:6, 2:82], AF.Silu,
                         scale=bdn2[32 * g:32 * g + 32, 0:1],
                         bias=bdn2[32 * g:32 * g + 32, 1:2])
                # edge masks: rows 2..5 (j0 g0 rows 0..3, et),
                # rows 46..49 (j1 g3 rows 2..5, eb)
                if j == 0:
                    nc.vector.tensor_scalar_mul(down[32:64, 2:6, :],
                                                down[32:64, 2:6, :], et[32:64])
                else:
                    nc.vector.tensor_scalar_mul(down[32:64, 46:50, :],
                                                down[32:64, 46:50, :], eb[32:64])


            # ---- S3: enc_cv1 (9 taps) + BN + SiLU -----------------------
            e1 = e[0:18]
            e1s_t = early.tile([128, 10 * WP + 8], BF16)
            nc.gpsimd.memset(e1s_t[:, 0:4], 0.0)
            nc.gpsimd.memset(e1s_t[:, 4 + 10 * WP:], 0.0)
            nc.vector.memset(e1[:, :, 0:2], 0.0)
            nc.vector.memset(e1[:, :, 82:84], 0.0)
            nc.vector.memset(e1[:, 2:3, 2:82], 0.0)
            nc.vector.memset(e1[:, 49:50, 2:82], 0.0)
            packed = set()
            for c0 in range(3, 49, 6):
                rr = min(6, 49 - c0)
                ps = psA.tile([128, 6, WP], F32, tag="ps")
                for t in range(9):
                    dh, dw = divmod(t, 3)
                    off = 4 + (c0 - 1 + dh) * WP + (dw - 1)
                    nc.tensor.matmul(
                        ps[0:18, 0:rr, :], wenc[:, t, :],
                        win(down_t, 0, 64, off, rr, WP),
                        start=(t == 0), stop=(t == 8))
                _act(nc, e[0:18, c0:c0 + rr, 2:82], ps[0:18, 0:rr, 2:82], AF.Silu,
                     scale=benc1[:, 0:1], bias=benc1[:, 1:2])
                if c0 == 3:
                    nc.vector.tensor_scalar_mul(e1[:, 3:6, :], e1[:, 3:6, :],
                                                et[0:18])
                if c0 == 45:
                    nc.vector.tensor_scalar_mul(e1[:, 46:49, :], e1[:, 46:49, :],
                                                eb[0:18])
                row_end = c0 + rr if c0 != 45 else 52
                for g in range(7):
                    if g not in packed and 6 * g + 13 <= row_end:
                        packed.add(g)
                        nc.sync.dma_start(
                            out=e1s_t[18 * g:18 * g + 18, 4:4 + 10 * WP],
                            in_=e1[:, 6 * g + 3:6 * g + 13, :])

            # ---- S4: enc_cv2 (diag bf16, 7 row-groups packed) -----------
            # group g (partitions 18g..18g+18) holds e1 rows [6g+3, 6g+13);
            # outputs rows [6g+5, 6g+11)
            ps = psA.tile([128, 6, WP], F32, tag="ps")
            for t in range(25):
                dh, dw = divmod(t, 5)
                off = 4 + dh * WP + (dw - 2)
                nc.tensor.matmul(
                    ps[0:126, 0:6, :], denc2p[:, t, :],
                    win(e1s_t, 0, 126, off, 6, WP),
                    start=(t == 0), stop=(t == 24))
            st = stage.tile([126, 6, WP], BF16, tag="ste2")
            _act(nc, st[:], ps[0:126, :, :], AF.Silu,
                 scale=benc2[:, 0:1], bias=benc2[:, 1:2])
            for g in range(7):
                nc.sync.dma_start(out=e[18:36, 6 * g + 5:6 * g + 11, :],
                                  in_=st[18 * g:18 * g + 18, :, :])

            # ztsall[w, zr, c] = zc[c, zr, w]  (one xbar DMA, 2 chunks);
            # deferred here so the copies don't contend with S2/S3 staging
            nc.scalar.dma_start_transpose(out=ztsall[:, 0:22, :], in_=zc[:, 0:22, :])
            nc.scalar.dma_start_transpose(out=ztsall[:, 22:NZ, :], in_=zc[:, 22:NZ, :])
            # dw-shifted copies at partition base 0 (zts_dw[i] = image col i+dw-1)
            for dw in range(3):
                nc.scalar.dma_start(out=zts[dw][:, :, :], in_=ztsall[dw + 1:dw + 85, :, :])

            # ---- S5: softmax -> ktn (pixel-major, batched) --------------
            # exp in channel-major (pixel j at col j), xbar-transpose to
            # pixel-major, reduce over k, normalize with values dup'd x2.
            RHC = [(0, 2), (2, 6), (8, 10), (18, 12), (30, 12)]
            for r0, rn in RHC:
                _act(nc, ebf[0:36, r0:r0 + rn, 0:82],
                     e[0:36, 5 + r0:5 + r0 + rn, 2:84], AF.Exp)
                nc.sync.dma_start_transpose(out=ktp[:, r0:r0 + rn, :],
                                            in_=ebf[:, r0:r0 + rn, :])
                nc.vector.tensor_reduce(
                    s[0:80, r0:r0 + rn, :],
                    ktp[0:80, r0:r0 + rn, 0:36].rearrange(
                        "w rh (k r) -> w rh r k", k=9),
                    axis=AX.X, op=ALU.add)
                nc.vector.reciprocal(
                    s[0:80, r0:r0 + rn, :].rearrange("w rh r -> w (rh r)"),
                    s[0:80, r0:r0 + rn, :].rearrange("w rh r -> w (rh r)"))
                kv = ktp[0:80, r0:r0 + rn, 0:36].rearrange(
                    "w rh (dh dw r) -> w rh dw dh r", dh=3, dw=3)
                neng = nc.vector if r0 < 12 else nc.gpsimd
                for dw in range(3):
                    for r in range(4):
                        neng.tensor_tensor(
                            ktn[0:80, r0:r0 + rn, dw, :, r, :],
                            kv[:, :, dw, :, r].unsqueeze(3)
                                .to_broadcast((80, rn, 3, 2)),
                            s[0:80, r0:r0 + rn, r:r + 1].unsqueeze(3)
                                .to_broadcast((80, rn, 3, 2)),
                            op=ALU.mult)

            # deferred big weights for phase B (issued last on the queue)
            nc.scalar.dma_start(out=bout1h[:], in_=d["bout1h_d"][:])
            nc.sync.dma_start(out=bout2[:], in_=d["bout2_d"][:])

        # early pool freed here
        with tc.tile_pool(name="late", bufs=1) as late:
            o2bf_t = late.tile([128, HO * WO + 8], BF16)
            o2bf = o2bf_t[:, 4:4 + HO * WO].rearrange("p (r w) -> p r w", w=WO)
            o2f8 = late.tile([64, 2, HO * WO + 8], F8)
            o2f8v = o2f8[:, :, 4:4 + HO * WO].rearrange("p g (r w) -> p g r w", w=WO)
            nc.gpsimd.memset(o2bf_t[:, 0:4], 0.0)
            nc.gpsimd.memset(o2bf_t[:, 4 + HO * WO:], 0.0)
            nc.gpsimd.memset(o2bf[:, :, 0:2], 0.0)
            nc.gpsimd.memset(o2bf[:, :, 162:164], 0.0)
            nc.gpsimd.memset(o2f8[:, :, 0:4], 0.0)
            nc.gpsimd.memset(o2f8[:, :, 4 + HO * WO:], 0.0)
            nc.gpsimd.memset(o2f8v[:, :, :, 0:2], 0.0)
            nc.gpsimd.memset(o2f8v[:, :, :, 162:164], 0.0)

            def win(flat, p0, p1, off, rr, w):
                return flat[p0:p1, off:off + rr * w].rearrange(
                    "p (r w) -> p r w", w=w)

            # S9 chunks: (c0, gr); ready when o2bf rows < c0+gr+2 exist.
            # PE chunks emit whole (matmuls are cheap); DVE/Pool chunks
            # emit S9_TAP_BUDGET taps per rh so B-builds aren't starved.
            qs = {"pe": [], "dve": [], "pool": []}
            ci = 0
            for c0 in range(2, 82, 3):
                gr = min(3, 82 - c0)
                eng = "dve" if ci in S9_DVE else ("pool" if ci in S9_POOL else "pe")
                qs[eng].append((c0, gr))
                ci += 1
            state = {"dve": None, "pool": None}   # in-flight: [c0, gr, t, acc]

            def finish_s9(c0, gr, src):
                st = stage.tile([128, 3, WO], F32, tag="st9")
                _act(nc, st[:, 0:gr, :], src, AF.Silu,
                     scale=bout2[:, 0:1], bias=bout2[:, 1:2])
                nc.sync.dma_start(out=out2_d[:, c0 - 2:c0 - 2 + gr, :],
                                  in_=st[:, 0:gr, 2:162])

            pe_state = {}   # in-flight PE chunk: [c0, gr, dh, ps]

            def step_s9_pe(ready_row):
                """Emit all PE dh-tap-groups whose input rows exist."""
                while True:
                    if not pe_state:
                        if not qs["pe"] or qs["pe"][0][0] + qs["pe"][0][1] - 2 \
                                > ready_row:
                            return
                        c0, gr = qs["pe"].pop(0)
                        ps9 = psA.tile([128, 3, WO], F32, tag="ps")
                        pe_state.update(c0=c0, gr=gr, dh=0, ps=ps9)
                    c0, gr = pe_state["c0"], pe_state["gr"]
                    dh = pe_state["dh"]
                    if c0 + gr - 2 + dh > ready_row:
                        return
                    ps = pe_state["ps"]
                    for dw in range(5):
                        t = dh * 5 + dw
                        off = 4 + (c0 - 2 + dh) * WO + (dw - 2)
                        nc.tensor.matmul(
                            ps[:, 0:gr, :], dgf8[:, t, :, :],
                            o2f8[:, :, off:off + gr * WO].rearrange(
                                "p g (r w) -> p g r w", w=WO),
                            start=(t == 0), stop=(t == 24),
                            perf_mode=PM.DoubleRow)
                    pe_state["dh"] = dh + 1
                    if pe_state["dh"] == 5:
                        finish_s9(c0, gr, ps[:, 0:gr, :])
                        pe_state.clear()

            def step_s9(eng, ready_row, budget):
                """Emit up to `budget` taps of eng's in-flight chunk."""
                v = nc.vector if eng == "dve" else nc.gpsimd
                while budget > 0:
                    if state[eng] is None:
                        if not qs[eng] or qs[eng][0][0] + qs[eng][0][1] + 2 > ready_row:
                            return
                        c0, gr = qs[eng].pop(0)
                        acc = stage.tile([128, 3, WO], F32, tag="acc" + eng)
                        state[eng] = [c0, gr, 0, acc]
                    c0, gr, t, acc = state[eng]
                    n = min(budget, 25 - t)
                    for tt in range(t, t + n):
                        dh, dw = divmod(tt, 5)
                        off = 4 + (c0 - 2 + dh) * WO + (dw - 2)
                        src_w = win(o2bf_t, 0, 128, off, gr, WO)
                        if tt == 0:
                            v.tensor_scalar_mul(acc[:, 0:gr, :], src_w,
                                                wto2[:, 0:1])
                        else:
                            v.scalar_tensor_tensor(
                                out=acc[:, 0:gr, :], in0=src_w,
                                scalar=wto2[:, tt:tt + 1], in1=acc[:, 0:gr, :],
                                op0=ALU.mult, op1=ALU.add)
                    budget -= n
                    state[eng][2] = t + n
                    if state[eng][2] == 25:
                        finish_s9(c0, gr, acc[:, 0:gr, :])
                        state[eng] = None

            next_s10 = 0       # S10 row groups of 8: g0 = 8*next_s10

            # ---- S7 + interleaved S9/S10 --------------------------------
            for rh in range(NKT):
                # build all 36 kt-scaled diagonal matrices in one DVE op:
                # B[w, dw, dh, r, wo] = ident[w, wo] * ktn[w, rh, 3*dh+dw, r]
                B = bpool.tile([128, 3, 3, 4, 80], BF16, tag="B")
                for dw in range(3):
                    eng = nc.gpsimd if (dw == 2 and rh >= 8) else nc.vector
                    eng.tensor_tensor(
                        B[0:80, dw].rearrange("w dh r (wp q) -> w (dh r) wp q", q=2),
                        identb[0:80, 0:80].rearrange("w (wp q) -> w wp q", q=2)
                            .unsqueeze(1).to_broadcast((80, 12, 40, 2)),
                        ktn[0:80, rh, dw].rearrange("w dh r q -> w (dh r) q")
                            .unsqueeze(2).to_broadcast((80, 12, 40, 2)),
                        op=ALU.mult)
                pt = psT.tile([128, 4, 80], F32, tag="pt")
                k = 0
                for dw in range(3):
                    for dh in range(3):
                        nc.tensor.matmul(
                            pt[:, :, :], zts[dw][0:80, rh + dh, :],
                            B[0:80, dw, dh, :, :],
                            start=(k == 0), stop=(k == 8))
                        k += 1
                _act(nc,
                     o2bf[:, 2 * rh:2 * rh + 2, 2:162].rearrange(
                         "p a (w b) -> p a b w", b=2),
                     pt[:, :, :].rearrange("p (a b) w -> p a b w", b=2),
                     AF.Silu,
                     scale=bout1[:, 0:1], bias=bout1[:, 1:2])
                for g in range(2):
                    _act(nc,
                         o2f8v[:, g, 2 * rh:2 * rh + 2, 2:162].rearrange(
                             "p a (w b) -> p a b w", b=2),
                         pt[64 * g:64 * g + 64, :, :].rearrange(
                             "p (a b) w -> p a b w", b=2),
                         AF.Silu,
                         scale=bout1h[:, g, 0:1], bias=bout1h[:, g, 1:2])
                if rh == 0:
                    nc.vector.tensor_scalar_mul(o2bf[:, 0:2, :], o2bf[:, 0:2, :], et)
                    for g in range(2):
                        nc.vector.tensor_copy(o2f8v[:, g, 0:2, :],
                                              o2bf[64 * g:64 * g + 64, 0:2, :])
                if rh == NKT - 1:
                    nc.vector.tensor_scalar_mul(o2bf[:, 82:84, :], o2bf[:, 82:84, :], eb)
                    for g in range(2):
                        nc.vector.tensor_copy(o2f8v[:, g, 82:84, :],
                                              o2bf[64 * g:64 * g + 64, 82:84, :])

                # interleave ready S9 work / S10 output DMAs
                ready_row = 2 * rh + 2
                step_s9_pe(ready_row)
                step_s9("dve", ready_row, S9_TAP_BUDGET)
                step_s9("pool", ready_row, S9_TAP_BUDGET)
                while next_s10 < 10 and 8 * next_s10 + 10 <= ready_row:
                    g0 = 8 * next_s10
                    nc.sync.dma_start(out=out1_d[:, g0:g0 + 8, :],
                                      in_=o2bf[:, 2 + g0:10 + g0, 2:162])
                    next_s10 += 1

            # drain remaining S9 work
            while qs["pe"] or pe_state:
                step_s9_pe(HO)
            while qs["dve"] or state["dve"] is not None:
                step_s9("dve", 84, 25)
            while qs["pool"] or state["pool"] is not None:
                step_s9("pool", 84, 25)
            while next_s10 < 10:
                g0 = 8 * next_s10
                nc.sync.dma_start(out=out1_d[:, g0:g0 + 8, :],
                                  in_=o2bf[:, 2 + g0:10 + g0, 2:162])
                next_s10 += 1


# ---------------------------------------------------------------------------
# host side
# ---------------------------------------------------------------------------

_NC_CACHE = {}


def _get_nc():
    if "nc" not in _NC_CACHE:
        _NC_CACHE["nc"] = build_kernel()
    return _NC_CACHE["nc"]


def _bn2(g, b, m, v):
    inv = (g / np.sqrt(v + EPS)).astype(np.float32)
    beta = (b - m * inv).astype(np.float32)
    return np.stack([inv, beta], axis=1).astype(np.float32)


def _diag_taps(w, c, rep=1):
    taps = np.tile(w.reshape(c, 25).T, (1, rep))      # (25, c*rep)
    n = c * rep
    out = np.zeros((25, n, n), np.float32)
    idx = np.arange(n)
    out[:, idx, idx] = taps
    return out.astype(ml_dtypes.bfloat16)


def _tile_bn(bn, rep):
    return np.tile(bn, (rep, 1))


def _wenc_pair(w):
    # taps (dh, dw) of the 3x3 conv; [0:64, dh] = (dh, 0), [64:128, dh] = (dh, 1),
    # [64:128, 3+dh] = (dh, 2) for the shifted-replica pairing in S3
    t = w.reshape(18, 64, 3, 3).transpose(2, 3, 1, 0)   # (dh, dw, c, m)
    out = np.zeros((128, 6, 18), np.float32)
    for dh in range(3):
        out[0:64, dh] = t[dh, 0]
        out[64:128, dh] = t[dh, 1]
        out[64:128, 3 + dh] = t[dh, 2]
    return out.astype(ml_dtypes.bfloat16)


def _dgf8(w):
    taps = w.reshape(128, 25).astype(np.float32)
    out = np.zeros((64, 25, 2, 128), np.float32)
    j = np.arange(64)
    for g in range(2):
        out[j[:, None], np.arange(25)[None, :], g, (64 * g + j)[:, None]] = \
            taps[64 * g:64 * g + 64, :]
    return out.astype(ml_dtypes.float8_e4m3)


def prep_in_maps(inputs):
    inp = {k: np.asarray(v) for k, v in inputs.items()}
    x = inp["x"].astype(np.float32)

    common = dict(
        wdn1=np.ascontiguousarray(inp["down_cv1_w"].reshape(32, 256).T).astype(ml_dtypes.bfloat16),
        bdn1=_bn2(inp["down_cv1_g"], inp["down_cv1_b"], inp["down_cv1_m"], inp["down_cv1_v"]),
        ddn2=_diag_taps(inp["down_cv2_w"], 32, rep=4),
        bdn2=_tile_bn(_bn2(inp["down_cv2_g"], inp["down_cv2_b"], inp["down_cv2_m"], inp["down_cv2_v"]), 4),
        wenc=np.ascontiguousarray(inp["enc_cv1_w"].reshape(18, 64, 9).transpose(2, 1, 0)).astype(ml_dtypes.bfloat16),
        benc1=_bn2(inp["enc_cv1_g"], inp["enc_cv1_b"], inp["enc_cv1_m"], inp["enc_cv1_v"]),
        denc2=_diag_taps(inp["enc_cv2_w"], 18, rep=7),
        benc2=_tile_bn(_bn2(inp["enc_cv2_g"], inp["enc_cv2_b"], inp["enc_cv2_m"], inp["enc_cv2_v"]), 7),
        wout1=np.ascontiguousarray(inp["out_cv1_w"].reshape(128, 256).T).astype(ml_dtypes.bfloat16),
        bout1=_bn2(inp["out_cv1_g"], inp["out_cv1_b"], inp["out_cv1_m"], inp["out_cv1_v"]),
        dgf8=_dgf8(inp["out_cv2_w"]),
        bout1h=_bn2(inp["out_cv1_g"], inp["out_cv1_b"], inp["out_cv1_m"], inp["out_cv1_v"])
            .reshape(2, 64, 2).transpose(1, 0, 2).copy(),
        wto2=np.ascontiguousarray(inp["out_cv2_w"].reshape(128, 25)).astype(np.float32),
        bout2=_bn2(inp["out_cv2_g"], inp["out_cv2_b"], inp["out_cv2_m"], inp["out_cv2_v"]),
    )

    in_maps = []
    for sid in range(8):
        n, half = sid // 2, sid % 2
        h0 = 40 * half
        xs = np.zeros((256, ROWS, WP), ml_dtypes.bfloat16)
        src_lo = max(0, h0 - 6)
        src_hi = min(80, h0 + 46)
        xs[:, src_lo - (h0 - 6):src_hi - (h0 - 6), 2:82] = x[n, :, src_lo:src_hi, :]
        edge = np.zeros((128, 2), np.float32)
        edge[:, 0] = 0.0 if half == 0 else 1.0
        edge[:, 1] = 1.0 if half == 0 else 0.0
        in_maps.append(dict(x=xs, edge=edge, **common))
    return in_maps


def kernel(**inputs):
    in_maps = prep_in_maps(inputs)
    nc = _get_nc()
    res = run_bass_kernel_spmd(nc, in_maps, list(range(8)))
    _NC_CACHE["last_result"] = res

    out = np.empty((4, 256, 160, 160), np.float32)
    for sid in range(8):
        n, half = sid // 2, sid % 2
        out[n, 0:128, 80 * half:80 * half + 80, :] = \
            res.results[sid]["out1"].astype(np.float32)
        out[n, 128:256, 80 * half:80 * half + 80, :] = res.results[sid]["out2"]
    return out


# revision 57
# speedup vs baseline: 1.0303x; 1.0192x over previous
"""CARAFE ghost-conv kernel for 8 Trainium2 NeuronCores.

Self-contained: takes FULL inputs (as in setup_inputs()), returns FULL output
(4, 256, 160, 160) float32.

Sharding: 8 cores = 4 batches x 2 H-halves (data-parallel, halo'd on host).
Per core: input rows [40*half-6, 40*half+46) (52 rows, zero-padded outside the
image), W padded 80->84 (cols 2..81 valid). Output rows [80*half, 80*half+80).

Pipeline per core (all resident in SBUF):
  S1 down_cv1 1x1 256->32 (PE) + BN+SiLU (ACT)
  S6 Z = out_cv1-conv(x) at low res (PE; CARAFE fused through the 1x1),
     DMA-transposed to pixel-major ztsall, then 3 dw-shifted copies zts1..3
  S2 down_cv2 dw5x5 (PE diag bf16) + BN+SiLU
  S3 enc_cv1 3x3 64->18 (PE, 9 shifted-tap matmuls) + BN+SiLU
  S4 enc_cv2 dw5x5 (PE diag bf16) + BN+SiLU
  S5 softmax: exp on ACT (channel-major) -> DMA-transpose to pixel-major ktp
     -> k-reduce + recip + normalize (DVE) -> ktn (values duplicated x2)
  S7 CARAFE row-pair rh: ONE DVE tensor_tensor builds 9x4 kt-scaled diagonal
     matrices B (ident x ktn broadcast); 9 PE matmuls (stationary zts row,
     moving B slice) accumulate all 4 phases into PSUM [128,4,80];
     ACT SiLU+BN -> o2c; GPSIMD cast -> o2bf
  S9 out_cv2 dw5x5 on the 160-grid, 3-row chunks interleaved into the S7
     loop, split across PE (diag bf16 matmuls) / DVE / GPSIMD (STT chains)
  S10 channels 0..127 output DMA, chunked + interleaved
"""

import numpy as np
import ml_dtypes

import concourse.bacc as bacc
import concourse.bass as bass
import concourse.tile as tile
from concourse import mybir
from concourse.bass_utils import run_bass_kernel_spmd
from concourse.masks import make_identity

F32 = mybir.dt.float32
F32R = mybir.dt.float32r
F8 = mybir.dt.float8e4
PM = mybir.MatmulPerfMode
BF16 = mybir.dt.bfloat16
AF = mybir.ActivationFunctionType
ALU = mybir.AluOpType
AX = mybir.AxisListType

EPS = 1e-5
WP = 84          # padded low-res width
ROWS = 52        # local input rows (valid image rows at local 6..45)
NKT = 42         # kt / o rows (local rows 5..46)
NZ = 44          # Z rows kept (local rows 4..47)
WO = 164         # padded hi-res width
HO = 84          # hi-res rows (output rows 80*half-2 .. 80*half+82)
WZ = 128         # zc padded width (DMA-transpose needs %128 free dim)

# S9 engine split: chunk index (of 27 3-row chunks) -> engine
S9_DVE = {3, 8, 13, 18}
S9_POOL = set()
S9_TAP_BUDGET = 3       # DVE/Pool taps emitted per rh iteration per engine


def _act(nc, out, in_, func, scale=1.0, bias=0.0):
    nc.scalar.activation(out=out, in_=in_, func=func, scale=scale, bias=bias)


def build_kernel():
    nc = bacc.Bacc("TRN2", target_bir_lowering=False, debug=False, num_devices=8)

    d = {}
    d["x_d"] = nc.declare_dram_parameter("x", [256, ROWS, WP], BF16, isOutput=False)
    d["edge_d"] = nc.declare_dram_parameter("edge", [128, 2], F32, isOutput=False)
    d["wdn1_d"] = nc.declare_dram_parameter("wdn1", [256, 32], BF16, isOutput=False)
    d["bdn1_d"] = nc.declare_dram_parameter("bdn1", [32, 2], F32, isOutput=False)
    d["ddn2_d"] = nc.declare_dram_parameter("ddn2", [25, 128, 128], BF16, isOutput=False)
    d["bdn2_d"] = nc.declare_dram_parameter("bdn2", [128, 2], F32, isOutput=False)
    d["wenc_d"] = nc.declare_dram_parameter("wenc", [9, 64, 18], BF16, isOutput=False)
    d["benc1_d"] = nc.declare_dram_parameter("benc1", [18, 2], F32, isOutput=False)
    d["denc2_d"] = nc.declare_dram_parameter("denc2", [25, 126, 126], BF16, isOutput=False)
    d["benc2_d"] = nc.declare_dram_parameter("benc2", [126, 2], F32, isOutput=False)
    d["wout1_d"] = nc.declare_dram_parameter("wout1", [256, 128], BF16, isOutput=False)
    d["bout1_d"] = nc.declare_dram_parameter("bout1", [128, 2], F32, isOutput=False)
    d["dgf8_d"] = nc.declare_dram_parameter("dgf8", [64, 25, 2, 128], F8, isOutput=False)
    d["bout1h_d"] = nc.declare_dram_parameter("bout1h", [64, 2, 2], F32, isOutput=False)
    d["wto2_d"] = nc.declare_dram_parameter("wto2", [128, 25], F32, isOutput=False)
    d["bout2_d"] = nc.declare_dram_parameter("bout2", [128, 2], F32, isOutput=False)
    d["out1_d"] = nc.declare_dram_parameter("out1", [128, 80, 160], BF16, isOutput=True)
    d["out2_d"] = nc.declare_dram_parameter("out2", [128, 80, 160], F32, isOutput=True)

    with tile.TileContext(nc) as tc:
        _emit(nc, tc, d)
    nc.compile()
    return nc


def _emit(nc, tc, d):
    x_d, out1_d, out2_d = d["x_d"], d["out1_d"], d["out2_d"]

    from contextlib import ExitStack
    ctx = ExitStack()
    with ctx:
        consts = ctx.enter_context(tc.tile_pool(name="consts", bufs=1))
        mid = ctx.enter_context(tc.tile_pool(name="mid", bufs=1))
        stage = ctx.enter_context(tc.tile_pool(name="stage", bufs=3))
        bpool = ctx.enter_context(tc.tile_pool(name="bpool", bufs=3))
        psA = ctx.enter_context(tc.tile_pool(name="psA", bufs=4, space="PSUM"))
        psT = ctx.enter_context(tc.tile_pool(name="psT", bufs=4, space="PSUM"))

        # ---- const tiles --------------------------------------------------
        ident = consts.tile([128, 128], F32)
        identb = consts.tile([128, 128], BF16)
        edge = consts.tile([128, 2], F32)
        wdn1 = consts.tile([128, 2, 32], BF16)
        bdn1 = consts.tile([32, 2], F32)
        ddn2p = consts.tile([128, 25, 128], BF16)
        bdn2 = consts.tile([128, 2], F32)
        wenc = consts.tile([64, 9, 18], BF16)
        benc1 = consts.tile([18, 2], F32)
        denc2p = consts.tile([126, 25, 126], BF16)
        benc2 = consts.tile([126, 2], F32)
        wout1 = consts.tile([128, 2, 128], BF16)
        bout1 = consts.tile([128, 2], F32)
        dgf8 = consts.tile([64, 25, 2, 128], F8)
        bout1h = consts.tile([64, 2, 2], F32)
        wto2 = consts.tile([128, 25], F32)
        bout2 = consts.tile([128, 2], F32)

        # mid-lived tensors
        ktn = mid.tile([128, NKT, 3, 3, 4, 2], BF16)  # [w,rh,dw,dh,r,dup]
        zt1 = mid.tile([84, NZ, 128], BF16)
        zt2 = mid.tile([84, NZ, 128], BF16)
        zt3 = mid.tile([84, NZ, 128], BF16)
        zts = {0: zt1, 1: zt2, 2: zt3}              # dw -> shifted Z (pixel-major)
        et, eb = edge[:, 0:1], edge[:, 1:2]

        with tc.tile_pool(name="early", bufs=1) as early:
            earlyA_cm = tc.tile_pool(name="earlyA", bufs=1)
            earlyA = earlyA_cm.__enter__()
            x0 = earlyA.tile([128, ROWS, WP], BF16)
            x1 = earlyA.tile([128, ROWS, WP], BF16)
            # input DMAs first so S1 can start ASAP
            nc.sync.dma_start(out=x0[:], in_=x_d[0:128])
            # small consts next, big diag weights in stage-use order
            make_identity(nc, ident[:])
            nc.gpsimd.tensor_copy(identb[:], ident[:])
            nc.sync.dma_start(out=edge[:], in_=d["edge_d"][:])
            nc.sync.dma_start(out=wdn1[:, 0, :], in_=d["wdn1_d"][0:128, :])
            nc.sync.dma_start(out=wdn1[:, 1, :], in_=d["wdn1_d"][128:256, :])
            nc.sync.dma_start(out=bdn1[:], in_=d["bdn1_d"][:])
            nc.sync.dma_start(out=wout1[:, 0, :], in_=d["wout1_d"][0:128, :])
            nc.sync.dma_start(out=wout1[:, 1, :], in_=d["wout1_d"][128:256, :])
            nc.sync.dma_start(out=bout1[:], in_=d["bout1_d"][:])
            nc.sync.dma_start(out=x1[:], in_=x_d[128:256])
            nc.sync.dma_start(out=bdn2[:], in_=d["bdn2_d"][:])
            nc.sync.dma_start(out=benc1[:], in_=d["benc1_d"][:])
            nc.sync.dma_start(out=benc2[:], in_=d["benc2_d"][:])

            down_t = early.tile([64, ROWS * WP + 8], BF16)
            down = down_t[:, 4:4 + ROWS * WP].rearrange("p (r w) -> p r w", w=WP)
            e = early.tile([36, ROWS, WP], BF16)
            zc = early.tile([128, NZ, WZ], BF16)
            ztsall = early.tile([128, NZ, 128], BF16)
            ebf = early.tile([48, NKT, 128], BF16)
            ktp = early.tile([128, NKT, 48], BF16)
            s = early.tile([128, NKT, 4], F32)
            nc.gpsimd.memset(down_t[:, 0:4], 0.0)
            nc.gpsimd.memset(down_t[:, 4 + ROWS * WP:], 0.0)
            nc.gpsimd.memset(zc[:, :, WP:WZ], 0.0)
            nc.gpsimd.memset(ebf[32:48, :, :], 0.0)
            nc.gpsimd.memset(ebf[0:36, :, 82:128], 0.0)

            def win(flat, p0, p1, off, rr, w):
                return flat[p0:p1, off:off + rr * w].rearrange(
                    "p (r w) -> p r w", w=w)

            y1 = down[0:32]
            nc.vector.memset(down[32:64, 0:2, :], 0.0)
            nc.vector.memset(down[32:64, 50:52, :], 0.0)
            nc.vector.memset(down[32:64, 2:50, 0:2], 0.0)
            nc.vector.memset(down[32:64, 2:50, 82:84], 0.0)
            nc.vector.memset(y1[:, :, 0:2], 0.0)
            nc.vector.memset(y1[:, :, 82:84], 0.0)

            # ---- S1: down_cv1 + BN + SiLU -------------------------------
            for c0 in range(0, ROWS, 6):
                rr = min(6, ROWS - c0)
                ps = psA.tile([128, 6, WP], F32, tag="ps")
                nc.tensor.matmul(ps[0:32, 0:rr, :], wdn1[:, 0, :],
                                 x0[:, c0:c0 + rr, :], start=True, stop=False)
                nc.tensor.matmul(ps[0:32, 0:rr, :], wdn1[:, 1, :],
                                 x1[:, c0:c0 + rr, :], start=False, stop=True)
                _act(nc, y1[:, c0:c0 + rr, 2:82], ps[0:32, 0:rr, 2:82], AF.Silu,
                     scale=bdn1[:, 0:1], bias=bdn1[:, 1:2])
            nc.vector.tensor_scalar_mul(y1[:, 0:6, :], y1[:, 0:6, :], et[0:32])
            nc.vector.tensor_scalar_mul(y1[:, 46:52, :], y1[:, 46:52, :], eb[0:32])

            # ---- S6: Z (out_cv1 conv, no BN) + pixel-major transform ----
            for c0 in range(0, NZ, 6):
                rr = min(6, NZ - c0)
                ps = psA.tile([128, 6, WP], F32, tag="ps")
                nc.tensor.matmul(ps[:, 0:rr, :], wout1[:, 0, :],
                                 x0[:, 4 + c0:4 + c0 + rr, :], start=True, stop=False)
                nc.tensor.matmul(ps[:, 0:rr, :], wout1[:, 1, :],
                                 x1[:, 4 + c0:4 + c0 + rr, :], start=False, stop=True)
                _act(nc, zc[:, c0:c0 + rr, 0:WP], ps[:, 0:rr, :], AF.Copy)
            earlyA_cm.__exit__(None, None, None)

            nc.gpsimd.dma_start(out=ddn2p[:], in_=d["ddn2_d"][:].rearrange("t k m -> k t m"))
            nc.gpsimd.dma_start(out=wenc[:], in_=d["wenc_d"][:].rearrange("t k m -> k t m"))
            nc.gpsimd.dma_start(out=denc2p[:], in_=d["denc2_d"][:].rearrange("t k m -> k t m"))
            nc.gpsimd.dma_start(out=dgf8[:], in_=d["dgf8_d"][:])
            nc.gpsimd.dma_start(out=wto2[:], in_=d["wto2_d"][:])

            # ---- S2: down_cv2 (diag bf16, 4 row-groups packed) ----------
            # group g (partitions 32g..32g+32) holds y1 rows [12g, 12g+16);
            # its outputs are rows [12g+2, 12g+14)
            y1s_t = early.tile([128, 16 * WP + 8], BF16)
            nc.gpsimd.memset(y1s_t[:, 0:4], 0.0)
            nc.gpsimd.memset(y1s_t[:, 4 + 16 * WP:], 0.0)
            for g in range(4):
                nc.sync.dma_start(
                    out=y1s_t[32 * g:32 * g + 32, 4:4 + 16 * WP],
                    in_=down_t[0:32, 4 + 12 * g * WP:4 + (12 * g + 16) * WP])
            for j in range(2):
                ps = psA.tile([128, 6, WP], F32, tag="ps")
                for t in range(25):
                    dh, dw = divmod(t, 5)
                    off = 4 + (6 * j + dh) * WP + (dw - 2)
                    nc.tensor.matmul(
                        ps[:, 0:6, :], ddn2p[:, t, :],
                        win(y1s_t, 0, 128, off, 6, WP),
                        start=(t == 0), stop=(t == 24))
                for g in range(4):
                    _act(nc,
                         down[32:64, 12 * g + 2 + 6 * j:12 * g + 8 + 6 * j, 2:82],
                         ps[32 * g:32 * g + 32, 0:6, 2:82], AF.Silu,
                         scale=bdn2[32 * g:32 * g + 32, 0:1],
                         bias=bdn2[32 * g:32 * g + 32, 1:2])
                # edge masks: rows 2..5 (j0 g0 rows 0..3, et),
                # rows 46..49 (j1 g3 rows 2..5, eb)
                if j == 0:
                    nc.vector.tensor_scalar_mul(down[32:64, 2:6, :],
                                                down[32:64, 2:6, :], et[32:64])
                else:
                    nc.vector.tensor_scalar_mul(down[32:64, 46:50, :],
                                                down[32:64, 46:50, :], eb[32:64])


            # ---- S3: enc_cv1 (9 taps) + BN + SiLU -----------------------
            e1 = e[0:18]
            e1s_t = early.tile([128, 10 * WP + 8], BF16)
            nc.gpsimd.memset(e1s_t[:, 0:4], 0.0)
            nc.gpsimd.memset(e1s_t[:, 4 + 10 * WP:], 0.0)
            nc.vector.memset(e1[:, :, 0:2], 0.0)
            nc.vector.memset(e1[:, :, 82:84], 0.0)
            nc.vector.memset(e1[:, 2:3, 2:82], 0.0)
            nc.vector.memset(e1[:, 49:50, 2:82], 0.0)
            packed = set()
            for c0 in range(3, 49, 6):
                rr = min(6, 49 - c0)
                ps = psA.tile([128, 6, WP], F32, tag="ps")
                for t in range(9):
                    dh, dw = divmod(t, 3)
                    off = 4 + (c0 - 1 + dh) * WP + (dw - 1)
                    nc.tensor.matmul(
                        ps[0:18, 0:rr, :], wenc[:, t, :],
                        win(down_t, 0, 64, off, rr, WP),
                        start=(t == 0), stop=(t == 8))
                _act(nc, e[0:18, c0:c0 + rr, 2:82], ps[0:18, 0:rr, 2:82], AF.Silu,
                     scale=benc1[:, 0:1], bias=benc1[:, 1:2])
                if c0 == 3:
                    nc.vector.tensor_scalar_mul(e1[:, 3:6, :], e1[:, 3:6, :],
                                                et[0:18])
                if c0 == 45:
                    nc.vector.tensor_scalar_mul(e1[:, 46:49, :], e1[:, 46:49, :],
                                                eb[0:18])
                row_end = c0 + rr if c0 != 45 else 52
                for g in range(7):
                    if g not in packed and 6 * g + 13 <= row_end:
                        packed.add(g)
                        nc.sync.dma_start(
                            out=e1s_t[18 * g:18 * g + 18, 4:4 + 10 * WP],
                            in_=e1[:, 6 * g + 3:6 * g + 13, :])

            # ---- S4: enc_cv2 (diag bf16, 7 row-groups packed) -----------
            # group g (partitions 18g..18g+18) holds e1 rows [6g+3, 6g+13);
            # outputs rows [6g+5, 6g+11)
            ps = psA.tile([128, 6, WP], F32, tag="ps")
            for t in range(25):
                dh, dw = divmod(t, 5)
                off = 4 + dh * WP + (dw - 2)
                nc.tensor.matmul(
                    ps[0:126, 0:6, :], denc2p[:, t, :],
                    win(e1s_t, 0, 126, off, 6, WP),
                    start=(t == 0), stop=(t == 24))
            st = stage.tile([126, 6, WP], BF16, tag="ste2")
            _act(nc, st[:], ps[0:126, :, :], AF.Silu,
                 scale=benc2[:, 0:1], bias=benc2[:, 1:2])
            for g in range(7):
                nc.sync.dma_start(out=e[18:36, 6 * g + 5:6 * g + 11, :],
                                  in_=st[18 * g:18 * g + 18, :, :])

            # ztsall[w, zr, c] = zc[c, zr, w]  (one xbar DMA, 2 chunks);
            # deferred here so the copies don't contend with S2/S3 staging
            nc.scalar.dma_start_transpose(out=ztsall[:, 0:22, :], in_=zc[:, 0:22, :])
            nc.scalar.dma_start_transpose(out=ztsall[:, 22:NZ, :], in_=zc[:, 22:NZ, :])
            # dw-shifted copies at partition base 0 (zts_dw[i] = image col i+dw-1)
            for dw in range(3):
                nc.scalar.dma_start(out=zts[dw][:, :, :], in_=ztsall[dw + 1:dw + 85, :, :])

            # ---- S5: softmax -> ktn (pixel-major, batched) --------------
            # exp in channel-major (pixel j at col j), xbar-transpose to
            # pixel-major, reduce over k, normalize with values dup'd x2.
            # rows 0..1: per-row PE transpose + ACT exp (skips the xbar DMA
            # latency on the critical S4 -> S7 pipeline fill)
            for rh in range(8):
                pt5 = psT.tile([80, 36], BF16, tag="pt")
                nc.tensor.transpose(pt5[:], e[0:36, 5 + rh, 2:82],
                                    identb[0:36, 0:36])
                _act(nc, ktp[0:80, rh, 0:36], pt5[:], AF.Exp)
            RHC = [(0, 2), (2, 6), (8, 10), (18, 12), (30, 12)]
            for ci, (r0, rn) in enumerate(RHC):
                if ci > 1:
                    _act(nc, ebf[0:36, r0:r0 + rn, 0:82],
                         e[0:36, 5 + r0:5 + r0 + rn, 2:84], AF.Exp)
                    nc.sync.dma_start_transpose(out=ktp[:, r0:r0 + rn, :],
                                                in_=ebf[:, r0:r0 + rn, :])
                nc.vector.tensor_reduce(
                    s[0:80, r0:r0 + rn, :],
                    ktp[0:80, r0:r0 + rn, 0:36].rearrange(
                        "w rh (k r) -> w rh r k", k=9),
                    axis=AX.X, op=ALU.add)
                nc.vector.reciprocal(
                    s[0:80, r0:r0 + rn, :].rearrange("w rh r -> w (rh r)"),
                    s[0:80, r0:r0 + rn, :].rearrange("w rh r -> w (rh r)"))
                kv = ktp[0:80, r0:r0 + rn, 0:36].rearrange(
                    "w rh (dh dw r) -> w rh dw dh r", dh=3, dw=3)
                neng = nc.vector if r0 < 12 else nc.gpsimd
                for dw in range(3):
                    for r in range(4):
                        neng.tensor_tensor(
                            ktn[0:80, r0:r0 + rn, dw, :, r, :],
                            kv[:, :, dw, :, r].unsqueeze(3)
                                .to_broadcast((80, rn, 3, 2)),
                            s[0:80, r0:r0 + rn, r:r + 1].unsqueeze(3)
                                .to_broadcast((80, rn, 3, 2)),
                            op=ALU.mult)

            # deferred big weights for phase B (issued last on the queue)
            nc.scalar.dma_start(out=bout1h[:], in_=d["bout1h_d"][:])
            nc.sync.dma_start(out=bout2[:], in_=d["bout2_d"][:])

        # early pool freed here
        with tc.tile_pool(name="late", bufs=1) as late:
            o2bf_t = late.tile([128, HO * WO + 8], BF16)
            o2bf = o2bf_t[:, 4:4 + HO * WO].rearrange("p (r w) -> p r w", w=WO)
            o2f8 = late.tile([64, 2, HO * WO + 8], F8)
            o2f8v = o2f8[:, :, 4:4 + HO * WO].rearrange("p g (r w) -> p g r w", w=WO)
            nc.gpsimd.memset(o2bf_t[:, 0:4], 0.0)
            nc.gpsimd.memset(o2bf_t[:, 4 + HO * WO:], 0.0)
            nc.gpsimd.memset(o2bf[:, :, 0:2], 0.0)
            nc.gpsimd.memset(o2bf[:, :, 162:164], 0.0)
            nc.gpsimd.memset(o2f8[:, :, 0:4], 0.0)
            nc.gpsimd.memset(o2f8[:, :, 4 + HO * WO:], 0.0)
            nc.gpsimd.memset(o2f8v[:, :, :, 0:2], 0.0)
            nc.gpsimd.memset(o2f8v[:, :, :, 162:164], 0.0)

            def win(flat, p0, p1, off, rr, w):
                return flat[p0:p1, off:off + rr * w].rearrange(
                    "p (r w) -> p r w", w=w)

            # S9 chunks: (c0, gr); ready when o2bf rows < c0+gr+2 exist.
            # PE chunks emit whole (matmuls are cheap); DVE/Pool chunks
            # emit S9_TAP_BUDGET taps per rh so B-builds aren't starved.
            qs = {"pe": [], "dve": [], "pool": []}
            ci = 0
            for c0 in range(2, 82, 3):
                gr = min(3, 82 - c0)
                eng = "dve" if ci in S9_DVE else ("pool" if ci in S9_POOL else "pe")
                qs[eng].append((c0, gr))
                ci += 1
            state = {"dve": None, "pool": None}   # in-flight: [c0, gr, t, acc]

            def finish_s9(c0, gr, src):
                st = stage.tile([128, 3, WO], F32, tag="st9")
                _act(nc, st[:, 0:gr, :], src, AF.Silu,
                     scale=bout2[:, 0:1], bias=bout2[:, 1:2])
                nc.sync.dma_start(out=out2_d[:, c0 - 2:c0 - 2 + gr, :],
                                  in_=st[:, 0:gr, 2:162])

            pe_state = {}   # in-flight PE chunk: [c0, gr, dh, ps]

            def step_s9_pe(ready_row):
                """Emit all PE dh-tap-groups whose input rows exist."""
                while True:
                    if not pe_state:
                        if not qs["pe"] or qs["pe"][0][0] + qs["pe"][0][1] - 2 \
                                > ready_row:
                            return
                        c0, gr = qs["pe"].pop(0)
                        ps9 = psA.tile([128, 3, WO], F32, tag="ps")
                        pe_state.update(c0=c0, gr=gr, dh=0, ps=ps9)
                    c0, gr = pe_state["c0"], pe_state["gr"]
                    dh = pe_state["dh"]
                    if c0 + gr - 2 + dh > ready_row:
                        return
                    ps = pe_state["ps"]
                    for dw in range(5):
                        t = dh * 5 + dw
                        off = 4 + (c0 - 2 + dh) * WO + (dw - 2)
                        nc.tensor.matmul(
                            ps[:, 0:gr, :], dgf8[:, t, :, :],
                            o2f8[:, :, off:off + gr * WO].rearrange(
                                "p g (r w) -> p g r w", w=WO),
                            start=(t == 0), stop=(t == 24),
                            perf_mode=PM.DoubleRow)
                    pe_state["dh"] = dh + 1
                    if pe_state["dh"] == 5:
                        finish_s9(c0, gr, ps[:, 0:gr, :])
                        pe_state.clear()

            def step_s9(eng, ready_row, budget):
                """Emit up to `budget` taps of eng's in-flight chunk."""
                v = nc.vector if eng == "dve" else nc.gpsimd
                while budget > 0:
                    if state[eng] is None:
                        if not qs[eng] or qs[eng][0][0] + qs[eng][0][1] + 2 > ready_row:
                            return
                        c0, gr = qs[eng].pop(0)
                        acc = stage.tile([128, 3, WO], F32, tag="acc" + eng)
                        state[eng] = [c0, gr, 0, acc]
                    c0, gr, t, acc = state[eng]
                    n = min(budget, 25 - t)
                    for tt in range(t, t + n):
                        dh, dw = divmod(tt, 5)
                        off = 4 + (c0 - 2 + dh) * WO + (dw - 2)
                        src_w = win(o2bf_t, 0, 128, off, gr, WO)
                        if tt == 0:
                            v.tensor_scalar_mul(acc[:, 0:gr, :], src_w,
                                                wto2[:, 0:1])
                        else:
                            v.scalar_tensor_tensor(
                                out=acc[:, 0:gr, :], in0=src_w,
                                scalar=wto2[:, tt:tt + 1], in1=acc[:, 0:gr, :],
                                op0=ALU.mult, op1=ALU.add)
                    budget -= n
                    state[eng][2] = t + n
                    if state[eng][2] == 25:
                        finish_s9(c0, gr, acc[:, 0:gr, :])
                        state[eng] = None

            next_s10 = 0       # S10 row groups of 8: g0 = 8*next_s10

            # ---- S7 + interleaved S9/S10 --------------------------------
            for rh in range(NKT):
                # build all 36 kt-scaled diagonal matrices in one DVE op:
                # B[w, dw, dh, r, wo] = ident[w, wo] * ktn[w, rh, 3*dh+dw, r]
                B = bpool.tile([128, 3, 3, 4, 80], BF16, tag="B")
                for dw in range(3):
                    eng = nc.gpsimd if (dw == 2 and rh >= 8) or (dw == 1 and rh >= 8 and rh % 3 == 0) else nc.vector
                    eng.tensor_tensor(
                        B[0:80, dw].rearrange("w dh r (wp q) -> w (dh r) wp q", q=2),
                        identb[0:80, 0:80].rearrange("w (wp q) -> w wp q", q=2)
                            .unsqueeze(1).to_broadcast((80, 12, 40, 2)),
                        ktn[0:80, rh, dw].rearrange("w dh r q -> w (dh r) q")
                            .unsqueeze(2).to_broadcast((80, 12, 40, 2)),
                        op=ALU.mult)
                pt = psT.tile([128, 4, 80], F32, tag="pt")
                k = 0
                for dw in range(3):
                    for dh in range(3):
                        nc.tensor.matmul(
                            pt[:, :, :], zts[dw][0:80, rh + dh, :],
                            B[0:80, dw, dh, :, :],
                            start=(k == 0), stop=(k == 8))
                        k += 1
                _act(nc,
                     o2bf[:, 2 * rh:2 * rh + 2, 2:162].rearrange(
                         "p a (w b) -> p a b w", b=2),
                     pt[:, :, :].rearrange("p (a b) w -> p a b w", b=2),
                     AF.Silu,
                     scale=bout1[:, 0:1], bias=bout1[:, 1:2])
                for g in range(2):
                    _act(nc,
                         o2f8v[:, g, 2 * rh:2 * rh + 2, 2:162].rearrange(
                             "p a (w b) -> p a b w", b=2),
                         pt[64 * g:64 * g + 64, :, :].rearrange(
                             "p (a b) w -> p a b w", b=2),
                         AF.Silu,
                         scale=bout1h[:, g, 0:1], bias=bout1h[:, g, 1:2])
                if rh == 0:
                    nc.vector.tensor_scalar_mul(o2bf[:, 0:2, :], o2bf[:, 0:2, :], et)
                    for g in range(2):
                        nc.vector.tensor_copy(o2f8v[:, g, 0:2, :],
                                              o2bf[64 * g:64 * g + 64, 0:2, :])
                if rh == NKT - 1:
                    nc.vector.tensor_scalar_mul(o2bf[:, 82:84, :], o2bf[:, 82:84, :], eb)
                    for g in range(2):
                        nc.vector.tensor_copy(o2f8v[:, g, 82:84, :],
                                              o2bf[64 * g:64 * g + 64, 82:84, :])

                # interleave ready S9 work / S10 output DMAs
                ready_row = 2 * rh + 2
                step_s9_pe(ready_row)
                step_s9("dve", ready_row, S9_TAP_BUDGET)
                step_s9("pool", ready_row, S9_TAP_BUDGET)
                while next_s10 < 10 and 8 * next_s10 + 10 <= ready_row:
                    g0 = 8 * next_s10
                    nc.sync.dma_start(out=out1_d[:, g0:g0 + 8, :],
                                      in_=o2bf[:, 2 + g0:10 + g0, 2:162])
                    next_s10 += 1

            # drain remaining S9 work
            while qs["pe"] or pe_state:
                step_s9_pe(HO)
            while qs["dve"] or state["dve"] is not None:
                step_s9("dve", 84, 25)
            while qs["pool"] or state["pool"] is not None:
                step_s9("pool", 84, 25)
            while next_s10 < 10:
                g0 = 8 * next_s10
                nc.sync.dma_start(out=out1_d[:, g0:g0 + 8, :],
                                  in_=o2bf[:, 2 + g0:10 + g0, 2:162])
                next_s10 += 1


# ---------------------------------------------------------------------------
# host side
# ---------------------------------------------------------------------------

_NC_CACHE = {}


def _get_nc():
    if "nc" not in _NC_CACHE:
        _NC_CACHE["nc"] = build_kernel()
    return _NC_CACHE["nc"]


def _bn2(g, b, m, v):
    inv = (g / np.sqrt(v + EPS)).astype(np.float32)
    beta = (b - m * inv).astype(np.float32)
    return np.stack([inv, beta], axis=1).astype(np.float32)


def _diag_taps(w, c, rep=1):
    taps = np.tile(w.reshape(c, 25).T, (1, rep))      # (25, c*rep)
    n = c * rep
    out = np.zeros((25, n, n), np.float32)
    idx = np.arange(n)
    out[:, idx, idx] = taps
    return out.astype(ml_dtypes.bfloat16)


def _tile_bn(bn, rep):
    return np.tile(bn, (rep, 1))


def _wenc_pair(w):
    # taps (dh, dw) of the 3x3 conv; [0:64, dh] = (dh, 0), [64:128, dh] = (dh, 1),
    # [64:128, 3+dh] = (dh, 2) for the shifted-replica pairing in S3
    t = w.reshape(18, 64, 3, 3).transpose(2, 3, 1, 0)   # (dh, dw, c, m)
    out = np.zeros((128, 6, 18), np.float32)
    for dh in range(3):
        out[0:64, dh] = t[dh, 0]
        out[64:128, dh] = t[dh, 1]
        out[64:128, 3 + dh] = t[dh, 2]
    return out.astype(ml_dtypes.bfloat16)


def _dgf8(w):
    taps = w.reshape(128, 25).astype(np.float32)
    out = np.zeros((64, 25, 2, 128), np.float32)
    j = np.arange(64)
    for g in range(2):
        out[j[:, None], np.arange(25)[None, :], g, (64 * g + j)[:, None]] = \
            taps[64 * g:64 * g + 64, :]
    return out.astype(ml_dtypes.float8_e4m3)


def prep_in_maps(inputs):
    inp = {k: np.asarray(v) for k, v in inputs.items()}
    x = inp["x"].astype(np.float32)

    common = dict(
        wdn1=np.ascontiguousarray(inp["down_cv1_w"].reshape(32, 256).T).astype(ml_dtypes.bfloat16),
        bdn1=_bn2(inp["down_cv1_g"], inp["down_cv1_b"], inp["down_cv1_m"], inp["down_cv1_v"]),
        ddn2=_diag_taps(inp["down_cv2_w"], 32, rep=4),
        bdn2=_tile_bn(_bn2(inp["down_cv2_g"], inp["down_cv2_b"], inp["down_cv2_m"], inp["down_cv2_v"]), 4),
        wenc=np.ascontiguousarray(inp["enc_cv1_w"].reshape(18, 64, 9).transpose(2, 1, 0)).astype(ml_dtypes.bfloat16),
        benc1=_bn2(inp["enc_cv1_g"], inp["enc_cv1_b"], inp["enc_cv1_m"], inp["enc_cv1_v"]),
        denc2=_diag_taps(inp["enc_cv2_w"], 18, rep=7),
        benc2=_tile_bn(_bn2(inp["enc_cv2_g"], inp["enc_cv2_b"], inp["enc_cv2_m"], inp["enc_cv2_v"]), 7),
        wout1=np.ascontiguousarray(inp["out_cv1_w"].reshape(128, 256).T).astype(ml_dtypes.bfloat16),
        bout1=_bn2(inp["out_cv1_g"], inp["out_cv1_b"], inp["out_cv1_m"], inp["out_cv1_v"]),
        dgf8=_dgf8(inp["out_cv2_w"]),
        bout1h=_bn2(inp["out_cv1_g"], inp["out_cv1_b"], inp["out_cv1_m"], inp["out_cv1_v"])
            .reshape(2, 64, 2).transpose(1, 0, 2).copy(),
        wto2=np.ascontiguousarray(inp["out_cv2_w"].reshape(128, 25)).astype(np.float32),
        bout2=_bn2(inp["out_cv2_g"], inp["out_cv2_b"], inp["out_cv2_m"], inp["out_cv2_v"]),
    )

    in_maps = []
    for sid in range(8):
        n, half = sid // 2, sid % 2
        h0 = 40 * half
        xs = np.zeros((256, ROWS, WP), ml_dtypes.bfloat16)
        src_lo = max(0, h0 - 6)
        src_hi = min(80, h0 + 46)
        xs[:, src_lo - (h0 - 6):src_hi - (h0 - 6), 2:82] = x[n, :, src_lo:src_hi, :]
        edge = np.zeros((128, 2), np.float32)
        edge[:, 0] = 0.0 if half == 0 else 1.0
        edge[:, 1] = 1.0 if half == 0 else 0.0
        in_maps.append(dict(x=xs, edge=edge, **common))
    return in_maps


def kernel(**inputs):
    in_maps = prep_in_maps(inputs)
    nc = _get_nc()
    res = run_bass_kernel_spmd(nc, in_maps, list(range(8)))
    _NC_CACHE["last_result"] = res

    out = np.empty((4, 256, 160, 160), np.float32)
    for sid in range(8):
        n, half = sid // 2, sid % 2
        out[n, 0:128, 80 * half:80 * half + 80, :] = \
            res.results[sid]["out1"].astype(np.float32)
        out[n, 128:256, 80 * half:80 * half + 80, :] = res.results[sid]["out2"]
    return out


# revision 66
# speedup vs baseline: 1.0357x; 1.0053x over previous
"""CARAFE ghost-conv kernel for 8 Trainium2 NeuronCores.

Self-contained: takes FULL inputs (as in setup_inputs()), returns FULL output
(4, 256, 160, 160) float32.

Sharding: 8 cores = 4 batches x 2 H-halves (data-parallel, halo'd on host).
Per core: input rows [40*half-6, 40*half+46) (52 rows, zero-padded outside the
image), W padded 80->84 (cols 2..81 valid). Output rows [80*half, 80*half+80).

Pipeline per core (all resident in SBUF):
  S1 down_cv1 1x1 256->32 (PE) + BN+SiLU (ACT)
  S6 Z = out_cv1-conv(x) at low res (PE; CARAFE fused through the 1x1),
     DMA-transposed to pixel-major ztsall, then 3 dw-shifted copies zts1..3
  S2 down_cv2 dw5x5 (PE diag bf16) + BN+SiLU
  S3 enc_cv1 3x3 64->18 (PE, 9 shifted-tap matmuls) + BN+SiLU
  S4 enc_cv2 dw5x5 (PE diag bf16) + BN+SiLU
  S5 softmax: exp on ACT (channel-major) -> DMA-transpose to pixel-major ktp
     -> k-reduce + recip + normalize (DVE) -> ktn (values duplicated x2)
  S7 CARAFE row-pair rh: ONE DVE tensor_tensor builds 9x4 kt-scaled diagonal
     matrices B (ident x ktn broadcast); 9 PE matmuls (stationary zts row,
     moving B slice) accumulate all 4 phases into PSUM [128,4,80];
     ACT SiLU+BN -> o2c; GPSIMD cast -> o2bf
  S9 out_cv2 dw5x5 on the 160-grid, 3-row chunks interleaved into the S7
     loop, split across PE (diag bf16 matmuls) / DVE / GPSIMD (STT chains)
  S10 channels 0..127 output DMA, chunked + interleaved
"""

import numpy as np
import ml_dtypes

import concourse.bacc as bacc
import concourse.bass as bass
import concourse.tile as tile
from concourse import mybir
from concourse.bass_utils import run_bass_kernel_spmd
from concourse.masks import make_identity

F32 = mybir.dt.float32
F32R = mybir.dt.float32r
F8 = mybir.dt.float8e4
PM = mybir.MatmulPerfMode
BF16 = mybir.dt.bfloat16
AF = mybir.ActivationFunctionType
ALU = mybir.AluOpType
AX = mybir.AxisListType

EPS = 1e-5
WP = 84          # padded low-res width
ROWS = 52        # local input rows (valid image rows at local 6..45)
NKT = 42         # kt / o rows (local rows 5..46)
NZ = 44          # Z rows kept (local rows 4..47)
WO = 164         # padded hi-res width
HO = 84          # hi-res rows (output rows 80*half-2 .. 80*half+82)
WZ = 128         # zc padded width (DMA-transpose needs %128 free dim)

# S9 engine split: chunk index (of 27 3-row chunks) -> engine
S9_DVE = {2, 7, 12, 17}
S9_POOL = set()
S9_TAP_BUDGET = 3       # DVE/Pool taps emitted per rh iteration per engine


def _act(nc, out, in_, func, scale=1.0, bias=0.0):
    nc.scalar.activation(out=out, in_=in_, func=func, scale=scale, bias=bias)


def build_kernel():
    nc = bacc.Bacc("TRN2", target_bir_lowering=False, debug=False, num_devices=8)

    d = {}
    d["x_d"] = nc.declare_dram_parameter("x", [256, ROWS, WP], BF16, isOutput=False)
    d["edge_d"] = nc.declare_dram_parameter("edge", [128, 2], F32, isOutput=False)
    d["wdn1_d"] = nc.declare_dram_parameter("wdn1", [256, 32], BF16, isOutput=False)
    d["bdn1_d"] = nc.declare_dram_parameter("bdn1", [32, 2], F32, isOutput=False)
    d["ddn2_d"] = nc.declare_dram_parameter("ddn2", [25, 128, 128], BF16, isOutput=False)
    d["bdn2_d"] = nc.declare_dram_parameter("bdn2", [128, 2], F32, isOutput=False)
    d["wenc_d"] = nc.declare_dram_parameter("wenc", [9, 64, 18], BF16, isOutput=False)
    d["benc1_d"] = nc.declare_dram_parameter("benc1", [18, 2], F32, isOutput=False)
    d["denc2_d"] = nc.declare_dram_parameter("denc2", [25, 126, 126], BF16, isOutput=False)
    d["benc2_d"] = nc.declare_dram_parameter("benc2", [126, 2], F32, isOutput=False)
    d["wout1_d"] = nc.declare_dram_parameter("wout1", [256, 128], BF16, isOutput=False)
    d["bout1_d"] = nc.declare_dram_parameter("bout1", [128, 2], F32, isOutput=False)
    d["dgf8_d"] = nc.declare_dram_parameter("dgf8", [64, 25, 2, 128], F8, isOutput=False)
    d["bout1h_d"] = nc.declare_dram_parameter("bout1h", [64, 2, 2], F32, isOutput=False)
    d["wto2_d"] = nc.declare_dram_parameter("wto2", [128, 25], F32, isOutput=False)
    d["bout2_d"] = nc.declare_dram_parameter("bout2", [128, 2], F32, isOutput=False)
    d["out1_d"] = nc.declare_dram_parameter("out1", [128, 80, 160], BF16, isOutput=True)
    d["out2_d"] = nc.declare_dram_parameter("out2", [128, 80, 160], F32, isOutput=True)

    with tile.TileContext(nc) as tc:
        _emit(nc, tc, d)
    nc.compile()
    return nc


def _emit(nc, tc, d):
    x_d, out1_d, out2_d = d["x_d"], d["out1_d"], d["out2_d"]

    from contextlib import ExitStack
    ctx = ExitStack()
    with ctx:
        consts = ctx.enter_context(tc.tile_pool(name="consts", bufs=1))
        mid = ctx.enter_context(tc.tile_pool(name="mid", bufs=1))
        stage = ctx.enter_context(tc.tile_pool(name="stage", bufs=3))
        bpool = ctx.enter_context(tc.tile_pool(name="bpool", bufs=3))
        psA = ctx.enter_context(tc.tile_pool(name="psA", bufs=4, space="PSUM"))
        psT = ctx.enter_context(tc.tile_pool(name="psT", bufs=4, space="PSUM"))

        # ---- const tiles --------------------------------------------------
        ident = consts.tile([128, 128], F32)
        identb = consts.tile([128, 128], BF16)
        edge = consts.tile([128, 2], F32)
        wdn1 = consts.tile([128, 2, 32], BF16)
        bdn1 = consts.tile([32, 2], F32)
        ddn2p = consts.tile([128, 25, 128], BF16)
        bdn2 = consts.tile([128, 2], F32)
        wenc = consts.tile([64, 9, 18], BF16)
        benc1 = consts.tile([18, 2], F32)
        denc2p = consts.tile([126, 25, 126], BF16)
        benc2 = consts.tile([126, 2], F32)
        wout1 = consts.tile([128, 2, 128], BF16)
        bout1 = consts.tile([128, 2], F32)
        dgf8 = consts.tile([64, 25, 2, 128], F8)
        bout1h = consts.tile([64, 2, 2], F32)
        wto2 = consts.tile([128, 25], F32)
        bout2 = consts.tile([128, 2], F32)

        # mid-lived tensors
        ktn = mid.tile([128, NKT, 3, 3, 4, 2], BF16)  # [w,rh,dw,dh,r,dup]
        zt1 = mid.tile([84, NZ, 128], BF16)
        zt2 = mid.tile([84, NZ, 128], BF16)
        zt3 = mid.tile([84, NZ, 128], BF16)
        zts = {0: zt1, 1: zt2, 2: zt3}              # dw -> shifted Z (pixel-major)
        et, eb = edge[:, 0:1], edge[:, 1:2]

        with tc.tile_pool(name="early", bufs=1) as early:
            earlyA_cm = tc.tile_pool(name="earlyA", bufs=1)
            earlyA = earlyA_cm.__enter__()
            x0 = earlyA.tile([128, ROWS, WP], BF16)
            x1 = earlyA.tile([128, ROWS, WP], BF16)
            # input DMAs first so S1 can start ASAP
            nc.sync.dma_start(out=x0[:], in_=x_d[0:128])
            # small consts next, big diag weights in stage-use order
            make_identity(nc, ident[:])
            nc.gpsimd.tensor_copy(identb[:], ident[:])
            nc.sync.dma_start(out=edge[:], in_=d["edge_d"][:])
            nc.sync.dma_start(out=wdn1[:, 0, :], in_=d["wdn1_d"][0:128, :])
            nc.sync.dma_start(out=wdn1[:, 1, :], in_=d["wdn1_d"][128:256, :])
            nc.sync.dma_start(out=bdn1[:], in_=d["bdn1_d"][:])
            nc.sync.dma_start(out=wout1[:, 0, :], in_=d["wout1_d"][0:128, :])
            nc.sync.dma_start(out=wout1[:, 1, :], in_=d["wout1_d"][128:256, :])
            nc.sync.dma_start(out=bout1[:], in_=d["bout1_d"][:])
            nc.sync.dma_start(out=x1[:], in_=x_d[128:256])
            nc.sync.dma_start(out=bdn2[:], in_=d["bdn2_d"][:])
            nc.sync.dma_start(out=benc1[:], in_=d["benc1_d"][:])
            nc.sync.dma_start(out=benc2[:], in_=d["benc2_d"][:])

            down_t = early.tile([64, ROWS * WP + 8], BF16)
            y1s_t = early.tile([128, 16 * WP + 8], BF16)
            nc.vector.memset(y1s_t[:, 0:4], 0.0)
            nc.vector.memset(y1s_t[:, 4 + 16 * WP:], 0.0)
            down = down_t[:, 4:4 + ROWS * WP].rearrange("p (r w) -> p r w", w=WP)
            e = early.tile([36, ROWS, WP], BF16)
            zc = early.tile([128, NZ, WZ], BF16)
            ztsall = early.tile([128, NZ, 128], BF16)
            ebf = early.tile([48, NKT, 128], BF16)
            ktp = early.tile([128, NKT, 48], BF16)
            s = early.tile([128, NKT, 4], F32)
            nc.gpsimd.memset(down_t[:, 0:4], 0.0)
            nc.gpsimd.memset(down_t[:, 4 + ROWS * WP:], 0.0)
            nc.gpsimd.memset(zc[:, :, WP:WZ], 0.0)
            nc.gpsimd.memset(ebf[32:48, :, :], 0.0)
            nc.gpsimd.memset(ebf[0:36, :, 82:128], 0.0)

            def win(flat, p0, p1, off, rr, w):
                return flat[p0:p1, off:off + rr * w].rearrange(
                    "p (r w) -> p r w", w=w)

            y1 = down[0:32]
            nc.vector.memset(down[32:64, 0:2, :], 0.0)
            nc.vector.memset(down[32:64, 50:52, :], 0.0)
            nc.vector.memset(down[32:64, 2:50, 0:2], 0.0)
            nc.vector.memset(down[32:64, 2:50, 82:84], 0.0)
            nc.vector.memset(y1[:, :, 0:2], 0.0)
            nc.vector.memset(y1[:, :, 82:84], 0.0)

            # ---- S1: down_cv1 + BN + SiLU -------------------------------
            packed1 = set()
            for c0 in range(0, ROWS, 6):
                rr = min(6, ROWS - c0)
                ps = psA.tile([128, 6, WP], F32, tag="ps")
                nc.tensor.matmul(ps[0:32, 0:rr, :], wdn1[:, 0, :],
                                 x0[:, c0:c0 + rr, :], start=True, stop=False)
                nc.tensor.matmul(ps[0:32, 0:rr, :], wdn1[:, 1, :],
                                 x1[:, c0:c0 + rr, :], start=False, stop=True)
                _act(nc, y1[:, c0:c0 + rr, 2:82], ps[0:32, 0:rr, 2:82], AF.Silu,
                     scale=bdn1[:, 0:1], bias=bdn1[:, 1:2])
                if c0 == 0:
                    nc.vector.tensor_scalar_mul(y1[:, 0:6, :], y1[:, 0:6, :],
                                                et[0:32])
                if c0 == 48:
                    nc.vector.tensor_scalar_mul(y1[:, 46:52, :], y1[:, 46:52, :],
                                                eb[0:32])
                row_end = 52 if c0 == 48 else c0 + rr
                for g in range(4):
                    if g not in packed1 and 12 * g + 16 <= row_end:
                        packed1.add(g)
                        nc.sync.dma_start(
                            out=y1s_t[32 * g:32 * g + 32, 4:4 + 16 * WP],
                            in_=down_t[0:32, 4 + 12 * g * WP:4 + (12 * g + 16) * WP])

            # ---- S6: Z (out_cv1 conv, no BN) + pixel-major transform ----
            for c0 in range(0, NZ, 6):
                rr = min(6, NZ - c0)
                ps = psA.tile([128, 6, WP], F32, tag="ps")
                nc.tensor.matmul(ps[:, 0:rr, :], wout1[:, 0, :],
                                 x0[:, 4 + c0:4 + c0 + rr, :], start=True, stop=False)
                nc.tensor.matmul(ps[:, 0:rr, :], wout1[:, 1, :],
                                 x1[:, 4 + c0:4 + c0 + rr, :], start=False, stop=True)
                _act(nc, zc[:, c0:c0 + rr, 0:WP], ps[:, 0:rr, :], AF.Copy)
            earlyA_cm.__exit__(None, None, None)

            nc.gpsimd.dma_start(out=ddn2p[:], in_=d["ddn2_d"][:].rearrange("t k m -> k t m"))
            nc.gpsimd.dma_start(out=wenc[:], in_=d["wenc_d"][:].rearrange("t k m -> k t m"))
            nc.gpsimd.dma_start(out=denc2p[:], in_=d["denc2_d"][:].rearrange("t k m -> k t m"))
            nc.gpsimd.dma_start(out=dgf8[:], in_=d["dgf8_d"][:])
            nc.gpsimd.dma_start(out=wto2[:], in_=d["wto2_d"][:])

            # ---- S2: down_cv2 (diag bf16, 4 row-groups packed) ----------
            # group g (partitions 32g..32g+32) holds y1 rows [12g, 12g+16);
            # its outputs are rows [12g+2, 12g+14)
            for j in range(2):
                ps = psA.tile([128, 6, WP], F32, tag="ps")
                for t in range(25):
                    dh, dw = divmod(t, 5)
                    off = 4 + (6 * j + dh) * WP + (dw - 2)
                    nc.tensor.matmul(
                        ps[:, 0:6, :], ddn2p[:, t, :],
                        win(y1s_t, 0, 128, off, 6, WP),
                        start=(t == 0), stop=(t == 24))
                for g in range(4):
                    _act(nc,
                         down[32:64, 12 * g + 2 + 6 * j:12 * g + 8 + 6 * j, 2:82],
                         ps[32 * g:32 * g + 32, 0:6, 2:82], AF.Silu,
                         scale=bdn2[32 * g:32 * g + 32, 0:1],
                         bias=bdn2[32 * g:32 * g + 32, 1:2])
                # edge masks: rows 2..5 (j0 g0 rows 0..3, et),
                # rows 46..49 (j1 g3 rows 2..5, eb)
                if j == 0:
                    nc.vector.tensor_scalar_mul(down[32:64, 2:6, :],
                                                down[32:64, 2:6, :], et[32:64])
                else:
                    nc.vector.tensor_scalar_mul(down[32:64, 46:50, :],
                                                down[32:64, 46:50, :], eb[32:64])


            # ---- S3: enc_cv1 (9 taps) + BN + SiLU -----------------------
            e1 = e[0:18]
            e1s_t = early.tile([128, 10 * WP + 8], BF16)
            nc.gpsimd.memset(e1s_t[:, 0:4], 0.0)
            nc.gpsimd.memset(e1s_t[:, 4 + 10 * WP:], 0.0)
            nc.vector.memset(e1[:, :, 0:2], 0.0)
            nc.vector.memset(e1[:, :, 82:84], 0.0)
            nc.vector.memset(e1[:, 2:3, 2:82], 0.0)
            nc.vector.memset(e1[:, 49:50, 2:82], 0.0)
            packed = set()
            for c0 in range(3, 49, 6):
                rr = min(6, 49 - c0)
                ps = psA.tile([128, 6, WP], F32, tag="ps")
                for t in range(9):
                    dh, dw = divmod(t, 3)
                    off = 4 + (c0 - 1 + dh) * WP + (dw - 1)
                    nc.tensor.matmul(
                        ps[0:18, 0:rr, :], wenc[:, t, :],
                        win(down_t, 0, 64, off, rr, WP),
                        start=(t == 0), stop=(t == 8))
                _act(nc, e[0:18, c0:c0 + rr, 2:82], ps[0:18, 0:rr, 2:82], AF.Silu,
                     scale=benc1[:, 0:1], bias=benc1[:, 1:2])
                if c0 == 3:
                    nc.vector.tensor_scalar_mul(e1[:, 3:6, :], e1[:, 3:6, :],
                                                et[0:18])
                if c0 == 45:
                    nc.vector.tensor_scalar_mul(e1[:, 46:49, :], e1[:, 46:49, :],
                                                eb[0:18])
                row_end = c0 + rr if c0 != 45 else 52
                for g in range(7):
                    if g not in packed and 6 * g + 13 <= row_end:
                        packed.add(g)
                        nc.sync.dma_start(
                            out=e1s_t[18 * g:18 * g + 18, 4:4 + 10 * WP],
                            in_=e1[:, 6 * g + 3:6 * g + 13, :])

            # ---- S4: enc_cv2 (diag bf16, 7 row-groups packed) -----------
            # group g (partitions 18g..18g+18) holds e1 rows [6g+3, 6g+13);
            # outputs rows [6g+5, 6g+11)
            ps = psA.tile([128, 6, WP], F32, tag="ps")
            for t in range(25):
                dh, dw = divmod(t, 5)
                off = 4 + dh * WP + (dw - 2)
                nc.tensor.matmul(
                    ps[0:126, 0:6, :], denc2p[:, t, :],
                    win(e1s_t, 0, 126, off, 6, WP),
                    start=(t == 0), stop=(t == 24))
            st = stage.tile([126, 6, WP], BF16, tag="ste2")
            _act(nc, st[:], ps[0:126, :, :], AF.Silu,
                 scale=benc2[:, 0:1], bias=benc2[:, 1:2])
            for g in range(7):
                nc.sync.dma_start(out=e[18:36, 6 * g + 5:6 * g + 11, :],
                                  in_=st[18 * g:18 * g + 18, :, :])

            # ztsall[w, zr, c] = zc[c, zr, w]  (one xbar DMA, 2 chunks);
            # deferred here so the copies don't contend with S2/S3 staging
            nc.scalar.dma_start_transpose(out=ztsall[:, 0:22, :], in_=zc[:, 0:22, :])
            nc.scalar.dma_start_transpose(out=ztsall[:, 22:NZ, :], in_=zc[:, 22:NZ, :])
            # dw-shifted copies at partition base 0 (zts_dw[i] = image col i+dw-1)
            for dw in range(3):
                nc.scalar.dma_start(out=zts[dw][:, :, :], in_=ztsall[dw + 1:dw + 85, :, :])

            # ---- S5: softmax -> ktn (pixel-major, batched) --------------
            # exp in channel-major (pixel j at col j), xbar-transpose to
            # pixel-major, reduce over k, normalize with values dup'd x2.
            # rows 0..1: per-row PE transpose + ACT exp (skips the xbar DMA
            # latency on the critical S4 -> S7 pipeline fill)
            for rh in range(8):
                pt5 = psT.tile([80, 36], BF16, tag="pt")
                nc.tensor.transpose(pt5[:], e[0:36, 5 + rh, 2:82],
                                    identb[0:36, 0:36])
                _act(nc, ktp[0:80, rh, 0:36], pt5[:], AF.Exp)
            RHC = [(0, 2), (2, 6), (8, 10), (18, 12), (30, 12)]
            for ci, (r0, rn) in enumerate(RHC):
                if ci > 1:
                    _act(nc, ebf[0:36, r0:r0 + rn, 0:82],
                         e[0:36, 5 + r0:5 + r0 + rn, 2:84], AF.Exp)
                    nc.sync.dma_start_transpose(out=ktp[:, r0:r0 + rn, :],
                                                in_=ebf[:, r0:r0 + rn, :])
                nc.vector.tensor_reduce(
                    s[0:80, r0:r0 + rn, :],
                    ktp[0:80, r0:r0 + rn, 0:36].rearrange(
                        "w rh (k r) -> w rh r k", k=9),
                    axis=AX.X, op=ALU.add)
                nc.vector.reciprocal(
                    s[0:80, r0:r0 + rn, :].rearrange("w rh r -> w (rh r)"),
                    s[0:80, r0:r0 + rn, :].rearrange("w rh r -> w (rh r)"))
                kv = ktp[0:80, r0:r0 + rn, 0:36].rearrange(
                    "w rh (dh dw r) -> w rh dw dh r", dh=3, dw=3)
                neng = nc.vector if r0 < 12 else nc.gpsimd
                for dw in range(3):
                    for r in range(4):
                        neng.tensor_tensor(
                            ktn[0:80, r0:r0 + rn, dw, :, r, :],
                            kv[:, :, dw, :, r].unsqueeze(3)
                                .to_broadcast((80, rn, 3, 2)),
                            s[0:80, r0:r0 + rn, r:r + 1].unsqueeze(3)
                                .to_broadcast((80, rn, 3, 2)),
                            op=ALU.mult)

            # deferred big weights for phase B (issued last on the queue)
            nc.scalar.dma_start(out=bout1h[:], in_=d["bout1h_d"][:])
            nc.sync.dma_start(out=bout2[:], in_=d["bout2_d"][:])

        # early pool freed here
        with tc.tile_pool(name="late", bufs=1) as late:
            o2bf_t = late.tile([128, HO * WO + 8], BF16)
            o2bf = o2bf_t[:, 4:4 + HO * WO].rearrange("p (r w) -> p r w", w=WO)
            o2f8 = late.tile([64, 2, HO * WO + 8], F8)
            o2f8v = o2f8[:, :, 4:4 + HO * WO].rearrange("p g (r w) -> p g r w", w=WO)
            nc.gpsimd.memset(o2bf_t[:, 0:4], 0.0)
            nc.gpsimd.memset(o2bf_t[:, 4 + HO * WO:], 0.0)
            nc.gpsimd.memset(o2bf[:, :, 0:2], 0.0)
            nc.gpsimd.memset(o2bf[:, :, 162:164], 0.0)
            nc.gpsimd.memset(o2f8[:, :, 0:4], 0.0)
            nc.gpsimd.memset(o2f8[:, :, 4 + HO * WO:], 0.0)
            nc.gpsimd.memset(o2f8v[:, :, :, 0:2], 0.0)
            nc.gpsimd.memset(o2f8v[:, :, :, 162:164], 0.0)

            def win(flat, p0, p1, off, rr, w):
                return flat[p0:p1, off:off + rr * w].rearrange(
                    "p (r w) -> p r w", w=w)

            # S9 chunks: (c0, gr); ready when o2bf rows < c0+gr+2 exist.
            # PE chunks emit whole (matmuls are cheap); DVE/Pool chunks
            # emit S9_TAP_BUDGET taps per rh so B-builds aren't starved.
            qs = {"pe": [], "dve": [], "pool": []}
            ci = 0
            for c0 in range(2, 82, 3):
                gr = min(3, 82 - c0)
                eng = "dve" if ci in S9_DVE else ("pool" if ci in S9_POOL else "pe")
                qs[eng].append((c0, gr))
                ci += 1
            state = {"dve": None, "pool": None}   # in-flight: [c0, gr, t, acc]

            def finish_s9(c0, gr, src):
                st = stage.tile([128, 3, WO], F32, tag="st9")
                _act(nc, st[:, 0:gr, :], src, AF.Silu,
                     scale=bout2[:, 0:1], bias=bout2[:, 1:2])
                nc.sync.dma_start(out=out2_d[:, c0 - 2:c0 - 2 + gr, :],
                                  in_=st[:, 0:gr, 2:162])

            pe_state = {}   # in-flight PE chunk: [c0, gr, dh, ps]

            def step_s9_pe(ready_row):
                """Emit all PE dh-tap-groups whose input rows exist."""
                while True:
                    if not pe_state:
                        if not qs["pe"] or qs["pe"][0][0] + qs["pe"][0][1] - 2 \
                                > ready_row:
                            return
                        c0, gr = qs["pe"].pop(0)
                        ps9 = psA.tile([128, 3, WO], F32, tag="ps")
                        pe_state.update(c0=c0, gr=gr, dh=0, ps=ps9)
                    c0, gr = pe_state["c0"], pe_state["gr"]
                    dh = pe_state["dh"]
                    if c0 + gr - 2 + dh > ready_row:
                        return
                    ps = pe_state["ps"]
                    for dw in range(5):
                        t = dh * 5 + dw
                        off = 4 + (c0 - 2 + dh) * WO + (dw - 2)
                        nc.tensor.matmul(
                            ps[:, 0:gr, :], dgf8[:, t, :, :],
                            o2f8[:, :, off:off + gr * WO].rearrange(
                                "p g (r w) -> p g r w", w=WO),
                            start=(t == 0), stop=(t == 24),
                            perf_mode=PM.DoubleRow)
                    pe_state["dh"] = dh + 1
                    if pe_state["dh"] == 5:
                        finish_s9(c0, gr, ps[:, 0:gr, :])
                        pe_state.clear()

            def step_s9(eng, ready_row, budget):
                """Emit up to `budget` taps of eng's in-flight chunk."""
                v = nc.vector if eng == "dve" else nc.gpsimd
                while budget > 0:
                    if state[eng] is None:
                        if not qs[eng] or qs[eng][0][0] + qs[eng][0][1] + 2 > ready_row:
                            return
                        c0, gr = qs[eng].pop(0)
                        acc = stage.tile([128, 3, WO], F32, tag="acc" + eng)
                        state[eng] = [c0, gr, 0, acc]
                    c0, gr, t, acc = state[eng]
                    n = min(budget, 25 - t)
                    for tt in range(t, t + n):
                        dh, dw = divmod(tt, 5)
                        off = 4 + (c0 - 2 + dh) * WO + (dw - 2)
                        src_w = win(o2bf_t, 0, 128, off, gr, WO)
                        if tt == 0:
                            v.tensor_scalar_mul(acc[:, 0:gr, :], src_w,
                                                wto2[:, 0:1])
                        else:
                            v.scalar_tensor_tensor(
                                out=acc[:, 0:gr, :], in0=src_w,
                                scalar=wto2[:, tt:tt + 1], in1=acc[:, 0:gr, :],
                                op0=ALU.mult, op1=ALU.add)
                    budget -= n
                    state[eng][2] = t + n
                    if state[eng][2] == 25:
                        finish_s9(c0, gr, acc[:, 0:gr, :])
                        state[eng] = None

            next_s10 = 0       # S10 row groups of 8: g0 = 8*next_s10

            # ---- S7 + interleaved S9/S10 --------------------------------
            for rh in range(NKT):
                # build all 36 kt-scaled diagonal matrices in one DVE op:
                # B[w, dw, dh, r, wo] = ident[w, wo] * ktn[w, rh, 3*dh+dw, r]
                B = bpool.tile([128, 3, 3, 4, 80], BF16, tag="B")
                for dw in range(3):
                    eng = nc.gpsimd if (dw == 2 and rh >= 8) or (dw == 1 and rh >= 8 and rh % 3 == 0) else nc.vector
                    eng.tensor_tensor(
                        B[0:80, dw].rearrange("w dh r (wp q) -> w (dh r) wp q", q=2),
                        identb[0:80, 0:80].rearrange("w (wp q) -> w wp q", q=2)
                            .unsqueeze(1).to_broadcast((80, 12, 40, 2)),
                        ktn[0:80, rh, dw].rearrange("w dh r q -> w (dh r) q")
                            .unsqueeze(2).to_broadcast((80, 12, 40, 2)),
                        op=ALU.mult)
                pt = psT.tile([128, 4, 80], F32, tag="pt")
                k = 0
                for dw in range(3):
                    for dh in range(3):
                        nc.tensor.matmul(
                            pt[:, :, :], zts[dw][0:80, rh + dh, :],
                            B[0:80, dw, dh, :, :],
                            start=(k == 0), stop=(k == 8))
                        k += 1
                _act(nc,
                     o2bf[:, 2 * rh:2 * rh + 2, 2:162].rearrange(
                         "p a (w b) -> p a b w", b=2),
                     pt[:, :, :].rearrange("p (a b) w -> p a b w", b=2),
                     AF.Silu,
                     scale=bout1[:, 0:1], bias=bout1[:, 1:2])
                for g in range(2):
                    _act(nc,
                         o2f8v[:, g, 2 * rh:2 * rh + 2, 2:162].rearrange(
                             "p a (w b) -> p a b w", b=2),
                         pt[64 * g:64 * g + 64, :, :].rearrange(
                             "p (a b) w -> p a b w", b=2),
                         AF.Silu,
                         scale=bout1h[:, g, 0:1], bias=bout1h[:, g, 1:2])
                if rh == 0:
                    nc.vector.tensor_scalar_mul(o2bf[:, 0:2, :], o2bf[:, 0:2, :], et)
                    for g in range(2):
                        nc.vector.tensor_copy(o2f8v[:, g, 0:2, :],
                                              o2bf[64 * g:64 * g + 64, 0:2, :])
                if rh == NKT - 1:
                    nc.vector.tensor_scalar_mul(o2bf[:, 82:84, :], o2bf[:, 82:84, :], eb)
                    for g in range(2):
                        nc.vector.tensor_copy(o2f8v[:, g, 82:84, :],
                                              o2bf[64 * g:64 * g + 64, 82:84, :])

                # interleave ready S9 work / S10 output DMAs
                ready_row = 2 * rh + 2
                step_s9_pe(ready_row)
                step_s9("dve", ready_row, S9_TAP_BUDGET)
                step_s9("pool", ready_row, S9_TAP_BUDGET)
                while next_s10 < 10 and 8 * next_s10 + 10 <= ready_row:
                    g0 = 8 * next_s10
                    nc.sync.dma_start(out=out1_d[:, g0:g0 + 8, :],
                                      in_=o2bf[:, 2 + g0:10 + g0, 2:162])
                    next_s10 += 1

            # drain remaining S9 work
            while qs["pe"] or pe_state:
                step_s9_pe(HO)
            while qs["dve"] or state["dve"] is not None:
                step_s9("dve", 84, 25)
            while qs["pool"] or state["pool"] is not None:
                step_s9("pool", 84, 25)
            while next_s10 < 10:
                g0 = 8 * next_s10
                nc.sync.dma_start(out=out1_d[:, g0:g0 + 8, :],
                                  in_=o2bf[:, 2 + g0:10 + g0, 2:162])
                next_s10 += 1


# ---------------------------------------------------------------------------
# host side
# ---------------------------------------------------------------------------

_NC_CACHE = {}


def _get_nc():
    if "nc" not in _NC_CACHE:
        _NC_CACHE["nc"] = build_kernel()
    return _NC_CACHE["nc"]


def _bn2(g, b, m, v):
    inv = (g / np.sqrt(v + EPS)).astype(np.float32)
    beta = (b - m * inv).astype(np.float32)
    return np.stack([inv, beta], axis=1).astype(np.float32)


def _diag_taps(w, c, rep=1):
    taps = np.tile(w.reshape(c, 25).T, (1, rep))      # (25, c*rep)
    n = c * rep
    out = np.zeros((25, n, n), np.float32)
    idx = np.arange(n)
    out[:, idx, idx] = taps
    return out.astype(ml_dtypes.bfloat16)


def _tile_bn(bn, rep):
    return np.tile(bn, (rep, 1))


def _wenc_pair(w):
    # taps (dh, dw) of the 3x3 conv; [0:64, dh] = (dh, 0), [64:128, dh] = (dh, 1),
    # [64:128, 3+dh] = (dh, 2) for the shifted-replica pairing in S3
    t = w.reshape(18, 64, 3, 3).transpose(2, 3, 1, 0)   # (dh, dw, c, m)
    out = np.zeros((128, 6, 18), np.float32)
    for dh in range(3):
        out[0:64, dh] = t[dh, 0]
        out[64:128, dh] = t[dh, 1]
        out[64:128, 3 + dh] = t[dh, 2]
    return out.astype(ml_dtypes.bfloat16)


def _dgf8(w):
    taps = w.reshape(128, 25).astype(np.float32)
    out = np.zeros((64, 25, 2, 128), np.float32)
    j = np.arange(64)
    for g in range(2):
        out[j[:, None], np.arange(25)[None, :], g, (64 * g + j)[:, None]] = \
            taps[64 * g:64 * g + 64, :]
    return out.astype(ml_dtypes.float8_e4m3)


def prep_in_maps(inputs):
    inp = {k: np.asarray(v) for k, v in inputs.items()}
    x = inp["x"].astype(np.float32)

    common = dict(
        wdn1=np.ascontiguousarray(inp["down_cv1_w"].reshape(32, 256).T).astype(ml_dtypes.bfloat16),
        bdn1=_bn2(inp["down_cv1_g"], inp["down_cv1_b"], inp["down_cv1_m"], inp["down_cv1_v"]),
        ddn2=_diag_taps(inp["down_cv2_w"], 32, rep=4),
        bdn2=_tile_bn(_bn2(inp["down_cv2_g"], inp["down_cv2_b"], inp["down_cv2_m"], inp["down_cv2_v"]), 4),
        wenc=np.ascontiguousarray(inp["enc_cv1_w"].reshape(18, 64, 9).transpose(2, 1, 0)).astype(ml_dtypes.bfloat16),
        benc1=_bn2(inp["enc_cv1_g"], inp["enc_cv1_b"], inp["enc_cv1_m"], inp["enc_cv1_v"]),
        denc2=_diag_taps(inp["enc_cv2_w"], 18, rep=7),
        benc2=_tile_bn(_bn2(inp["enc_cv2_g"], inp["enc_cv2_b"], inp["enc_cv2_m"], inp["enc_cv2_v"]), 7),
        wout1=np.ascontiguousarray(inp["out_cv1_w"].reshape(128, 256).T).astype(ml_dtypes.bfloat16),
        bout1=_bn2(inp["out_cv1_g"], inp["out_cv1_b"], inp["out_cv1_m"], inp["out_cv1_v"]),
        dgf8=_dgf8(inp["out_cv2_w"]),
        bout1h=_bn2(inp["out_cv1_g"], inp["out_cv1_b"], inp["out_cv1_m"], inp["out_cv1_v"])
            .reshape(2, 64, 2).transpose(1, 0, 2).copy(),
        wto2=np.ascontiguousarray(inp["out_cv2_w"].reshape(128, 25)).astype(np.float32),
        bout2=_bn2(inp["out_cv2_g"], inp["out_cv2_b"], inp["out_cv2_m"], inp["out_cv2_v"]),
    )

    in_maps = []
    for sid in range(8):
        n, half = sid // 2, sid % 2
        h0 = 40 * half
        xs = np.zeros((256, ROWS, WP), ml_dtypes.bfloat16)
        src_lo = max(0, h0 - 6)
        src_hi = min(80, h0 + 46)
        xs[:, src_lo - (h0 - 6):src_hi - (h0 - 6), 2:82] = x[n, :, src_lo:src_hi, :]
        edge = np.zeros((128, 2), np.float32)
        edge[:, 0] = 0.0 if half == 0 else 1.0
        edge[:, 1] = 1.0 if half == 0 else 0.0
        in_maps.append(dict(x=xs, edge=edge, **common))
    return in_maps


def kernel(**inputs):
    in_maps = prep_in_maps(inputs)
    nc = _get_nc()
    res = run_bass_kernel_spmd(nc, in_maps, list(range(8)))
    _NC_CACHE["last_result"] = res

    out = np.empty((4, 256, 160, 160), np.float32)
    for sid in range(8):
        n, half = sid // 2, sid % 2
        out[n, 0:128, 80 * half:80 * half + 80, :] = \
            res.results[sid]["out1"].astype(np.float32)
        out[n, 128:256, 80 * half:80 * half + 80, :] = res.results[sid]["out2"]
    return out
